# revision 2
# baseline (speedup 1.0000x reference)
"""Causal multi-head attention on 8 Trainium2 NeuronCores.

Problem: x[2,2048,1024] @ W_Q/K/V[1024,1024] -> 16-head causal attention
(d_head=64) -> @ W_O[1024,1024].

Sharding: core c = (hq, b) with hq = c//2, b = c%2. Core owns heads
4hq..4hq+3 (columns [256hq:256hq+256) of W_Q/K/V, rows of W_O) and batch b.
Each core computes a partial output [1024, 2048] (transposed, bf16); the
host sums the 4 head-quad partials per batch and un-transposes.

Design (per core, bf16 operands, fp32 PSUM):
  - Fused pipeline over 4 token tiles of 512: project tile t, causal
    attention for q-tile j=t (its K/V chunks are all ready), W_O for tile
    j - with projection/W_O matmuls of neighboring tiles rationed into
    the in-order PE queue between attention chunks so the PE stays busy
    (HAM stays warm) while the scalar engine (exp) paces the steady state.
  - Q/K projected transposed ([dims, tokens]); V projected directly in
    natural [token, dim] layout (lhsT = x chunk) - no PE transposes.
  - Heads packed in pairs on partitions 0-63 / 64-127; the two score
    matmuls of a pair run on disjoint PE row groups (concurrent on HW).
  - One exp per (pair, chunk) over [128, 2, 512] straddling two PSUM
    banks; causal band masking via gpsimd affine_select on the probs.
  - PV accumulates [96, 512] per head (64 dims + ones column + pad; the
    ones column yields the softmax denominator for free). Reciprocal via
    the fast custom DVE op on base-partition-0 staging tiles; the
    normalize broadcasts+multiply are deferred into the next sweep so
    they never head-of-line-block the Pool queue.
  - All DRAM layouts chunk-major so every DMA moves contiguous blocks.
PSUM: scores 2x[128,2,512] (4 banks) + PV [96,2,512] (2) + proj/WO ring
2x[128,512] (2) = 8 banks.

Measured: 377us (fp32r phase-serial baseline) -> ~181us, rel err 3.9e-3.
"""

from collections import deque

import numpy as np

import concourse.bass as bass
import concourse.tile as tile
from concourse import bacc, mybir
from concourse.bass_utils import run_bass_kernel_spmd

F32 = mybir.dt.float32
BF16 = mybir.dt.bfloat16

N_CORES = 8
P = 128

# HW-bisection flags (CoreSim passes all combinations; HW may not)
USE_STRIDED_EXP = True    # one exp over [128, 2, 512] vs per-head 2D exps
USE_AFFSEL = True         # causal mask via gpsimd.affine_select on probs
USE_FAST_RECIP = True     # reciprocal_approx_fast direct from PSUM
XPOOL_BUFS = 4            # x-tile ring depth; 2 races on HW (DMA vs PE reads)
D = 1024          # d_model
B = 2             # batch
S = 2048          # seq len per core (one batch)
TT = 512          # token tile (free dim of matmuls)
NT = S // TT      # 4 token tiles
KD = D // P       # 8 contraction chunks for projections
NCH = S // P      # 16 k-chunks
HD = 256          # head dims per core (4 heads x 64)
DH = 64           # head dim
NP = 2            # head pairs per core
VW = 96           # V block width: 64 dims + ones col + pad to a 32-multiple


def _body(tc):
    nc = tc.nc
    # all DRAM layouts are chunk-major so every DMA moves a contiguous
    # [128, *] block (strided descriptors halve DMA throughput)
    xT = nc.dram_tensor("xT", [KD, NT, P, TT], BF16, kind="ExternalInput").ap()
    wq = nc.dram_tensor("wq", [KD, P, HD], BF16, kind="ExternalInput").ap()
    wk = nc.dram_tensor("wk", [KD, P, HD], BF16, kind="ExternalInput").ap()
    wv = nc.dram_tensor("wv", [KD, P, HD], BF16, kind="ExternalInput").ap()
    wo = nc.dram_tensor("wo", [P, NP, D], BF16, kind="ExternalInput").ap()
    outT = nc.dram_tensor("outT", [KD, NT, P, TT], BF16,
                          kind="ExternalOutput").ap()

    import contextlib
    with contextlib.ExitStack() as ctx:
        wpool = ctx.enter_context(tc.tile_pool(name="wpool", bufs=1))
        xpool = ctx.enter_context(tc.tile_pool(name="xpool", bufs=XPOOL_BUFS))
        persist = ctx.enter_context(tc.tile_pool(name="persist", bufs=1))
        prp = ctx.enter_context(tc.tile_pool(name="probs", bufs=6))
        stage = ctx.enter_context(tc.tile_pool(name="stage", bufs=3))
        obp = ctx.enter_context(tc.tile_pool(name="obp", bufs=3))
        ps_sc = ctx.enter_context(tc.tile_pool(name="ps_sc", bufs=2, space="PSUM"))
        ps_pv = ctx.enter_context(tc.tile_pool(name="ps_pv", bufs=1, space="PSUM"))
        ps_pw = ctx.enter_context(tc.tile_pool(name="ps_pw", bufs=2, space="PSUM"))

        # mask_band[k, q] = 1.0 if q >= k else 0.0 (only for USE_AFFSEL=False)
        mask_band = None
        if not USE_AFFSEL:
            mask_band = wpool.tile([P, P], F32)
            nc.any.memset(mask_band[:], 1.0)
            nc.gpsimd.affine_select(
                out=mask_band[:], in_=mask_band[:],
                compare_op=mybir.AluOpType.is_ge,
                fill=0.0, base=0, pattern=[[1, P]],
                channel_multiplier=-1)

        # --- PE warmup: dummy matmuls keep the HAM clock-gate warm while
        # the initial DMAs stream in (no data deps, results discarded)
        dum = wpool.tile([P, TT], BF16)
        nc.vector.memset(dum[:], 0.0)
        dps = ps_pw.tile([P, TT], F32, tag="pw", name="warmup")
        for _ in range(14):
            nc.tensor.matmul(dps[:], dum[:, 0:P], dum[:],
                             start=True, stop=True)

        # --- weights (per-chunk DMAs so the first matmuls start early) ----
        xt0 = xpool.tile([P, KD, TT], BF16, tag="xt", name="xt_0")
        wq_sb = wpool.tile([P, KD, HD], BF16)
        wk_sb = wpool.tile([P, KD, HD], BF16)
        wv_sb = wpool.tile([P, KD, HD], BF16)
        for c in range(KD):
            nc.gpsimd.dma_start(xt0[:, c, :], xT[c, 0])
            nc.sync.dma_start(wq_sb[:, c, :], wq[c])
            nc.sync.dma_start(wk_sb[:, c, :], wk[c])
        for c in range(KD):   # wv only feeds the (deferred) V projection
            nc.sync.dma_start(wv_sb[:, c, :], wv[c])
        wo_sb = wpool.tile([P, NP, D], BF16)
        nc.sync.dma_start(wo_sb[:], wo)

        # --- persistent activations ---------------------------------------
        qT = persist.tile([P, NP, S], BF16)    # pair p: head 2p on part 0-63
        kT = persist.tile([P, NP, S], BF16)
        vn = persist.tile([P, NCH, 4, VW], BF16)  # [tok, chunk, head, d|1|pad]
        attnT = persist.tile([P, NP, S], BF16)
        # ones columns via memset (NOT activation(in*0+1): uninitialized SBUF
        # may hold NaN and NaN*0 = NaN on hardware); zero the pad columns
        for l in range(4):
            nc.vector.memset(vn[:, :, l, DH], 1.0)
            nc.vector.memset(vn[:, :, l, DH + 1:VW], 0.0)

        # deferred work: closures emitting ~1-2us of PE work each, drained
        # between attention chunk iterations to keep the PE queue dense.
        # Rationed so the queue lasts the whole attention phase (the PE
        # would otherwise run dry near pair transitions and HAM-rethrottle).
        work = deque()
        ration = [0.0, 0.0]   # step, accumulator

        def set_ration(iters, skip=0):
            ration[0] = len(work) / max(iters - skip, 1)
            ration[1] = -skip * ration[0]

        def drain_step():
            ration[1] += ration[0]
            while ration[1] >= 1.0 and work:
                work.popleft()()
                ration[1] -= 1.0

        def drain(n):
            for _ in range(min(n, len(work))):
                work.popleft()()

        def make_proj(t, xt):
            """Projection closures for token tile t (uses staged xt)."""
            ops = []
            tsl = bass.ts(t, TT)

            def qk_proj(wsb, dstT, p, t=t, xt=xt, tsl=tsl):
                def f():
                    ps = ps_pw.tile([P, TT], F32, tag="pw",
                                    name=f"pj_{wsb is wk_sb}_{t}_{p}")
                    for c in range(KD):
                        nc.tensor.matmul(ps[:], wsb[:, c, bass.ts(p, P)],
                                         xt[:, c, :],
                                         start=(c == 0), stop=(c == KD - 1))
                    nc.vector.tensor_copy(dstT[:, p, tsl], ps[:])
                return f

            def v_proj(tc0, t=t, xt=xt):
                def f():
                    ps = ps_pw.tile([P, 2, HD], F32, tag="pw",
                                    name=f"pv_{t}_{tc0}")
                    for i in range(2):
                        tci = tc0 + i
                        for c in range(KD):
                            nc.tensor.matmul(
                                ps[:, i, :],
                                xt[:, c, bass.ts(tci, P)],
                                wv_sb[:, c, :],
                                start=(c == 0), stop=(c == KD - 1))
                    for i in range(2):
                        ch = t * 4 + tc0 + i
                        nc.vector.tensor_copy(
                            vn[:, ch, :, 0:DH],
                            ps[:, i, :].rearrange("p (h d) -> p h d", h=4))
                return f

            for p in range(NP):
                ops.append(qk_proj(wq_sb, qT, p))
                ops.append(qk_proj(wk_sb, kT, p))
            ops.append(v_proj(0))
            ops.append(v_proj(2))
            return ops

        def make_wo(j):
            """Output projection closures for q-tile j (reads attnT)."""
            ops = []
            jsl = bass.ts(j, TT)
            for f_ in range(KD):
                def g(f_=f_, jsl=jsl, j=j):
                    ps = ps_pw.tile([P, TT], F32, tag="pw",
                                    name=f"wo_{j}_{f_}")
                    for a in range(NP):
                        nc.tensor.matmul(ps[:], wo_sb[:, a, bass.ts(f_, P)],
                                         attnT[:, a, jsl],
                                         start=(a == 0), stop=(a == NP - 1))
                    ob = obp.tile([P, TT], BF16, tag="ob", name=f"ob_{j}_{f_}")
                    nc.vector.tensor_copy(ob[:], ps[:])
                    nc.sync.dma_start(outT[f_, j], ob[:])
                ops.append(g)
            return ops

        def pair_sweep(j, p):
            """Scores+exp+mask+PV chunk loop for (q-tile j, head pair p),
            followed by the PSUM drain copies, reciprocal, and per-pair
            normalize of attnT."""
            jsl = bass.ts(j, TT)
            ncb = 4 * (j + 1)
            dinv = stage.tile([1, NP, TT], F32, tag=f"dinv{p}",
                              name=f"dinv_{j}_{p}")
            pv = ps_pv.tile([VW, NP, TT], F32, tag="pv", name=f"pv_{j}_{p}")
            pvh = [pv[:, h, :] for h in range(NP)]
            pending = None
            for cb in range(ncb):
                r = cb - 4 * j
                lo = P * r if r > 0 else 0
                csl = bass.ts(cb, P)
                sc = ps_sc.tile([P, NP, TT], F32, tag="sc",
                                name=f"sc_{j}_{p}_{cb}")
                for h in range(NP):
                    hp = slice(DH * h, DH * h + DH)
                    nc.tensor.matmul(sc[:, h, lo:], kT[hp, p, csl],
                                     qT[hp, p, jsl][:, lo:],
                                     start=True, stop=True)
                pr = prp.tile([P, NP, TT], BF16, tag="pr",
                              name=f"pr_{j}_{p}_{cb}")
                if USE_STRIDED_EXP:
                    nc.scalar.activation(pr[:, :, lo:], sc[:, :, lo:],
                                         mybir.ActivationFunctionType.Exp,
                                         scale=0.125)
                else:
                    for h in range(NP):
                        nc.scalar.activation(
                            pr[:, h, lo:], sc[:, h, lo:],
                            mybir.ActivationFunctionType.Exp, scale=0.125)
                if r >= 0:
                    rsl = bass.ts(r, P)
                    for h in range(NP):
                        if USE_AFFSEL:
                            nc.gpsimd.affine_select(
                                out=pr[:, h, rsl], in_=pr[:, h, rsl],
                                compare_op=mybir.AluOpType.is_ge,
                                fill=0.0, base=0, pattern=[[1, P]],
                                channel_multiplier=-1)
                        else:
                            nc.vector.tensor_mul(pr[:, h, rsl],
                                                 pr[:, h, rsl],
                                                 mask_band[:])
                if pending is not None:
                    pcb, ppr, plo = pending
                    for h in range(NP):
                        nc.tensor.matmul(
                            pvh[h][:, plo:],
                            vn[:, pcb, 2 * p + h, :], ppr[:, h, plo:],
                            start=(pcb == 0), stop=False)
                pending = (cb, pr, lo)
                if j == 0 and p == 0 and cb == 0:
                    drain(2)   # V closures must emit before the first PV
                drain_step()
            pcb, ppr, plo = pending
            for h in range(NP):
                nc.tensor.matmul(pvh[h][:, plo:],
                                 vn[:, pcb, 2 * p + h, :], ppr[:, h, plo:],
                                 start=(pcb == 0), stop=True)

            # free the PV banks: unnormalized copy + denominator recip
            dsb = stage.tile([1, NP, TT], F32, tag=f"dsb{p}",
                             name=f"dsb_{j}_{p}")
            for h in range(NP):
                nc.vector.tensor_copy(attnT[bass.ts(h, DH), p, jsl],
                                      pvh[h][0:DH, :])
                nc.vector.tensor_copy(dsb[0:1, h, :],
                                      pvh[h][DH:DH + 1, :])
            if USE_FAST_RECIP:
                nc.vector.reciprocal_approx_fast(out=dinv[:], in_=dsb[:])
            else:
                nc.vector.reciprocal(dinv[:], dsb[:])

            # normalize tail as a deferred closure: emitted a few chunk
            # iterations into the NEXT sweep, so the Pool-queue broadcasts
            # never sit at the queue head waiting on the reciprocal chain
            # (head-of-line blocking the next sweep's affine_selects).
            # partition_broadcast does not honor a non-zero output base
            # partition on HW (head1 goes via a base-0 temp + copy); DVE
            # TensorTensor needs equal input base partitions in SBUF.
            def norm_tail(j=j, p=p, jsl=jsl, dinv=dinv):
                rbf = stage.tile([P, TT], F32, tag="rbf", name=f"rbf_{j}_{p}")
                nc.gpsimd.partition_broadcast(rbf[0:DH, :], dinv[0:1, 0, :])
                rbt = stage.tile([DH, TT], F32, tag="rbt", name=f"rbt_{j}_{p}")
                nc.gpsimd.partition_broadcast(rbt[:], dinv[0:1, 1, :])
                nc.vector.tensor_copy(rbf[DH:P, :], rbt[:])
                nc.vector.tensor_mul(attnT[:, p, jsl], attnT[:, p, jsl],
                                     rbf[:])
            work.appendleft(norm_tail)

        # tile 0: q/k projections immediately; V closures drain into attn(0).
        # xt1 is staged up front too (its DMA shares the startup window),
        # with proj(1) closures drained during attn(0) pair 1.
        qk0 = make_proj(0, xt0)
        for f in qk0[:4]:
            f()
        work.extend(qk0[4:])
        xt1 = xpool.tile([P, KD, TT], BF16, tag="xt", name="xt_1")
        for c in range(KD):
            nc.gpsimd.dma_start(xt1[:, c, :], xT[c, 1])
        work.extend(make_proj(1, xt1))

        for j in range(NT):
            ncb = 4 * (j + 1)
            # stage + enqueue tile j+1 at phase start (xt1 staged upfront);
            # the ration spreads its closures over BOTH pair sweeps, with a
            # few skipped iterations up front so drained matmuls never wait
            # on the just-issued xt DMA in the in-order PE queue.
            if 1 <= j and j + 1 < NT:
                xt = xpool.tile([P, KD, TT], BF16, tag="xt", name=f"xt_{j+1}")
                for c in range(KD):
                    nc.gpsimd.dma_start(xt[:, c, :], xT[c, j + 1])
                work.extend(make_proj(j + 1, xt))

            # attn(0) pair 0 runs while xt1/wv still stream: only the V
            # closures (explicit drain) go in; everything else would block
            # the in-order PE queue behind unarrived DMAs. Later phases
            # spread the drains over both sweeps, skipping a few iterations
            # so drained matmuls never wait on the just-issued xt DMA.
            set_ration(2 * ncb if j > 0 else 10**9, skip=4 if j > 0 else 0)
            pair_sweep(j, 0)
            if j == 0:
                set_ration(ncb)
            pair_sweep(j, 1)
            drain(len(work))   # proj(j+1) fully emitted before attn(j+1)
            work.extend(make_wo(j))
        drain(len(work))


_NC_CACHE = None


def _get_nc():
    global _NC_CACHE
    if _NC_CACHE is None:
        nc = bacc.Bacc("TRN2", target_bir_lowering=False, debug=False,
                       num_devices=N_CORES)
        with tile.TileContext(nc) as tc:
            _body(tc)
        nc.compile()
        _NC_CACHE = nc
    return _NC_CACHE


def _in_maps(x, W_Q, W_K, W_V, W_O):
    import ml_dtypes
    bf = ml_dtypes.bfloat16
    x32 = np.asarray(x, dtype=np.float32)
    W_Q = np.asarray(W_Q, dtype=np.float32)
    W_K = np.asarray(W_K, dtype=np.float32)
    W_V = np.asarray(W_V, dtype=np.float32)
    W_O = np.asarray(W_O, dtype=np.float32)
    # chunk-major layouts so every device DMA is a contiguous block
    xTb = [np.ascontiguousarray(
        x32[b].T.reshape(KD, P, NT, TT).transpose(0, 2, 1, 3)).astype(bf)
        for b in range(B)]
    maps = []
    for core in range(N_CORES):
        hq, b = core // 2, core % 2
        sl = slice(HD * hq, HD * hq + HD)
        maps.append({
            "xT": xTb[b],
            "wq": np.ascontiguousarray(W_Q[:, sl].reshape(KD, P, HD)).astype(bf),
            "wk": np.ascontiguousarray(W_K[:, sl].reshape(KD, P, HD)).astype(bf),
            "wv": np.ascontiguousarray(W_V[:, sl].reshape(KD, P, HD)).astype(bf),
            "wo": np.ascontiguousarray(
                W_O[sl, :].reshape(NP, P, D).transpose(1, 0, 2)).astype(bf),
        })
    return maps


def _gather(results):
    acc = np.zeros([B, D, S], np.float32)
    for core, r in enumerate(results):
        hq, b = core // 2, core % 2
        acc[b] += r["outT"].astype(np.float32).transpose(
            0, 2, 1, 3).reshape(D, S)
    return np.ascontiguousarray(acc.transpose(0, 2, 1))


def kernel(x, W_Q, W_K, W_V, W_O):
    nc = _get_nc()
    res = run_bass_kernel_spmd(nc, _in_maps(x, W_Q, W_K, W_V, W_O),
                               core_ids=list(range(N_CORES)))
    return _gather(res.results)


def kernel_profiled(x, W_Q, W_K, W_V, W_O):
    """Like kernel() but with NTFF tracing; returns (output, exec_time_ns)."""
    nc = _get_nc()
    res = run_bass_kernel_spmd(nc, _in_maps(x, W_Q, W_K, W_V, W_O),
                               core_ids=list(range(N_CORES)), trace=True)
    return _gather(res.results), res.exec_time_ns


# revision 3
# speedup vs baseline: 1.0181x; 1.0181x over previous
"""Causal multi-head attention on 8 Trainium2 NeuronCores.

Problem: x[2,2048,1024] @ W_Q/K/V[1024,1024] -> 16-head causal attention
(d_head=64) -> @ W_O[1024,1024].

Sharding: core c = (hq, b) with hq = c//2, b = c%2. Core owns heads
4hq..4hq+3 (columns [256hq:256hq+256) of W_Q/K/V, rows of W_O) and batch b.
Each core computes a partial output [1024, 2048] (transposed, bf16); the
host sums the 4 head-quad partials per batch and un-transposes.

Design (per core, bf16 operands, fp32 PSUM):
  - Fused pipeline over 4 token tiles of 512: project tile t, causal
    attention for q-tile j=t (its K/V chunks are all ready), W_O for tile
    j - with projection/W_O matmuls of neighboring tiles rationed into
    the in-order PE queue between attention chunks so the PE stays busy
    (HAM stays warm) while the scalar engine (exp) paces the steady state.
  - Q/K projected transposed ([dims, tokens]); V projected directly in
    natural [token, dim] layout (lhsT = x chunk) - no PE transposes.
  - Heads packed in pairs on partitions 0-63 / 64-127; the two score
    matmuls of a pair run on disjoint PE row groups (concurrent on HW).
  - One exp per (pair, chunk) over [128, 2, 512] straddling two PSUM
    banks; causal band masking via gpsimd affine_select on the probs.
  - PV accumulates [96, 512] per head (64 dims + ones column + pad; the
    ones column yields the softmax denominator for free). Reciprocal via
    the fast custom DVE op on base-partition-0 staging tiles; the
    normalize broadcasts+multiply are deferred into the next sweep so
    they never head-of-line-block the Pool queue.
  - All DRAM layouts chunk-major so every DMA moves contiguous blocks.
PSUM: scores 2x[128,2,512] (4 banks) + PV [96,2,512] (2) + proj/WO ring
2x[128,512] (2) = 8 banks.

Measured: 377us (fp32r phase-serial baseline) -> ~181us, rel err 3.9e-3.
"""

from collections import deque

import numpy as np

import concourse.bass as bass
import concourse.tile as tile
from concourse import bacc, mybir
from concourse.bass_utils import run_bass_kernel_spmd

F32 = mybir.dt.float32
BF16 = mybir.dt.bfloat16

N_CORES = 8
P = 128

# HW-bisection flags (CoreSim passes all combinations; HW may not)
USE_STRIDED_EXP = True    # one exp over [128, 2, 512] vs per-head 2D exps
USE_AFFSEL = True         # causal mask via gpsimd.affine_select on probs
USE_FAST_RECIP = True     # reciprocal_approx_fast direct from PSUM
XPOOL_BUFS = 4            # x-tile ring depth; 2 races on HW (DMA vs PE reads)
D = 1024          # d_model
B = 2             # batch
S = 2048          # seq len per core (one batch)
TT = 512          # token tile (free dim of matmuls)
NT = S // TT      # 4 token tiles
KD = D // P       # 8 contraction chunks for projections
NCH = S // P      # 16 k-chunks
HD = 256          # head dims per core (4 heads x 64)
DH = 64           # head dim
NP = 2            # head pairs per core
VW = 96           # V block width: 64 dims + ones col + pad to a 32-multiple


def _body(tc):
    nc = tc.nc
    # all DRAM layouts are chunk-major so every DMA moves a contiguous
    # [128, *] block (strided descriptors halve DMA throughput)
    xT = nc.dram_tensor("xT", [KD, NT, P, TT], BF16, kind="ExternalInput").ap()
    wq = nc.dram_tensor("wq", [KD, P, HD], BF16, kind="ExternalInput").ap()
    wk = nc.dram_tensor("wk", [KD, P, HD], BF16, kind="ExternalInput").ap()
    wv = nc.dram_tensor("wv", [KD, P, HD], BF16, kind="ExternalInput").ap()
    wo = nc.dram_tensor("wo", [P, NP, D], BF16, kind="ExternalInput").ap()
    outT = nc.dram_tensor("outT", [KD, NT, P, TT], BF16,
                          kind="ExternalOutput").ap()

    import contextlib
    with contextlib.ExitStack() as ctx:
        wpool = ctx.enter_context(tc.tile_pool(name="wpool", bufs=1))
        xpool = ctx.enter_context(tc.tile_pool(name="xpool", bufs=XPOOL_BUFS))
        persist = ctx.enter_context(tc.tile_pool(name="persist", bufs=1))
        prp = ctx.enter_context(tc.tile_pool(name="probs", bufs=6))
        stage = ctx.enter_context(tc.tile_pool(name="stage", bufs=3))
        obp = ctx.enter_context(tc.tile_pool(name="obp", bufs=3))
        ps_sc = ctx.enter_context(tc.tile_pool(name="ps_sc", bufs=2, space="PSUM"))
        ps_pv = ctx.enter_context(tc.tile_pool(name="ps_pv", bufs=1, space="PSUM"))
        ps_pw = ctx.enter_context(tc.tile_pool(name="ps_pw", bufs=2, space="PSUM"))

        # mask_band[k, q] = 1.0 if q >= k else 0.0 (only for USE_AFFSEL=False)
        mask_band = None
        if not USE_AFFSEL:
            mask_band = wpool.tile([P, P], F32)
            nc.any.memset(mask_band[:], 1.0)
            nc.gpsimd.affine_select(
                out=mask_band[:], in_=mask_band[:],
                compare_op=mybir.AluOpType.is_ge,
                fill=0.0, base=0, pattern=[[1, P]],
                channel_multiplier=-1)

        # --- PE warmup: dummy matmuls keep the HAM clock-gate warm while
        # the initial DMAs stream in (no data deps, results discarded)
        dum = wpool.tile([P, TT], BF16)
        nc.vector.memset(dum[:], 0.0)
        dps = ps_pw.tile([P, TT], F32, tag="pw", name="warmup")
        for _ in range(14):
            nc.tensor.matmul(dps[:], dum[:, 0:P], dum[:],
                             start=True, stop=True)

        # --- weights (per-chunk DMAs so the first matmuls start early) ----
        xt0 = xpool.tile([P, KD, TT], BF16, tag="xt", name="xt_0")
        wq_sb = wpool.tile([P, KD, HD], BF16)
        wk_sb = wpool.tile([P, KD, HD], BF16)
        wv_sb = wpool.tile([P, KD, HD], BF16)
        for c in range(KD):
            nc.gpsimd.dma_start(xt0[:, c, :], xT[c, 0])
            nc.sync.dma_start(wq_sb[:, c, :], wq[c])
            nc.sync.dma_start(wk_sb[:, c, :], wk[c])
        for c in range(KD):   # wv only feeds the (deferred) V projection
            nc.sync.dma_start(wv_sb[:, c, :], wv[c])
        wo_sb = wpool.tile([P, NP, D], BF16)
        nc.sync.dma_start(wo_sb[:], wo)

        # --- persistent activations ---------------------------------------
        qT = persist.tile([P, NP, S], BF16)    # pair p: head 2p on part 0-63
        kT = persist.tile([P, NP, S], BF16)
        vn = persist.tile([P, NCH, 4, VW], BF16)  # [tok, chunk, head, d|1|pad]
        attnT = persist.tile([P, NP, S], BF16)
        # ones columns via memset (NOT activation(in*0+1): uninitialized SBUF
        # may hold NaN and NaN*0 = NaN on hardware); zero the pad columns
        for l in range(4):
            nc.vector.memset(vn[:, :, l, DH], 1.0)
            nc.vector.memset(vn[:, :, l, DH + 1:VW], 0.0)

        # deferred work: closures emitting ~1-2us of PE work each, drained
        # between attention chunk iterations to keep the PE queue dense.
        # Rationed so the queue lasts the whole attention phase (the PE
        # would otherwise run dry near pair transitions and HAM-rethrottle).
        work = deque()
        ration = [0.0, 0.0]   # step, accumulator

        def set_ration(iters, skip=0):
            ration[0] = len(work) / max(iters - skip, 1)
            ration[1] = -skip * ration[0]

        def drain_step():
            ration[1] += ration[0]
            while ration[1] >= 1.0 and work:
                work.popleft()()
                ration[1] -= 1.0

        def drain(n):
            for _ in range(min(n, len(work))):
                work.popleft()()

        def make_proj(t, xt):
            """Projection closures for token tile t (uses staged xt)."""
            ops = []
            tsl = bass.ts(t, TT)

            def qk_proj(wsb, dstT, p, t=t, xt=xt, tsl=tsl):
                def f():
                    ps = ps_pw.tile([P, TT], F32, tag="pw",
                                    name=f"pj_{wsb is wk_sb}_{t}_{p}")
                    for c in range(KD):
                        nc.tensor.matmul(ps[:], wsb[:, c, bass.ts(p, P)],
                                         xt[:, c, :],
                                         start=(c == 0), stop=(c == KD - 1))
                    nc.vector.tensor_copy(dstT[:, p, tsl], ps[:])
                return f

            def v_proj(tc0, t=t, xt=xt):
                def f():
                    ps = ps_pw.tile([P, 2, HD], F32, tag="pw",
                                    name=f"pv_{t}_{tc0}")
                    for i in range(2):
                        tci = tc0 + i
                        for c in range(KD):
                            nc.tensor.matmul(
                                ps[:, i, :],
                                xt[:, c, bass.ts(tci, P)],
                                wv_sb[:, c, :],
                                start=(c == 0), stop=(c == KD - 1))
                    for i in range(2):
                        ch = t * 4 + tc0 + i
                        nc.vector.tensor_copy(
                            vn[:, ch, :, 0:DH],
                            ps[:, i, :].rearrange("p (h d) -> p h d", h=4))
                return f

            for p in range(NP):
                ops.append(qk_proj(wq_sb, qT, p))
                ops.append(qk_proj(wk_sb, kT, p))
            ops.append(v_proj(0))
            ops.append(v_proj(2))
            return ops

        def make_wo(j):
            """Output projection closures for q-tile j (reads attnT)."""
            ops = []
            jsl = bass.ts(j, TT)
            # the last tile's W_O runs after attention ends: borrow the
            # score-PSUM ring (free by then) for 2x the groups in flight
            wpool_ps = ps_sc if j == NT - 1 else ps_pw
            wtag = "sc" if j == NT - 1 else "pw"
            for f_ in range(KD):
                def g(f_=f_, jsl=jsl, j=j):
                    ps = wpool_ps.tile([P, TT], F32, tag=wtag,
                                       name=f"wo_{j}_{f_}")
                    for a in range(NP):
                        nc.tensor.matmul(ps[:], wo_sb[:, a, bass.ts(f_, P)],
                                         attnT[:, a, jsl],
                                         start=(a == 0), stop=(a == NP - 1))
                    ob = obp.tile([P, TT], BF16, tag="ob", name=f"ob_{j}_{f_}")
                    if j == NT - 1 and f_ % 2:
                        # tail: ACT is idle after the last exp — share the
                        # PSUM drain copies between the two engines
                        nc.scalar.activation(
                            ob[:], ps[:], mybir.ActivationFunctionType.Copy)
                    else:
                        nc.vector.tensor_copy(ob[:], ps[:])
                    nc.sync.dma_start(outT[f_, j], ob[:])
                ops.append(g)
            return ops

        def pair_sweep(j, p):
            """Scores+exp+mask+PV chunk loop for (q-tile j, head pair p),
            followed by the PSUM drain copies, reciprocal, and per-pair
            normalize of attnT."""
            jsl = bass.ts(j, TT)
            ncb = 4 * (j + 1)
            dinv = stage.tile([1, NP, TT], F32, tag=f"dinv{p}",
                              name=f"dinv_{j}_{p}")
            pv = ps_pv.tile([VW, NP, TT], F32, tag="pv", name=f"pv_{j}_{p}")
            pvh = [pv[:, h, :] for h in range(NP)]
            pending = None
            for cb in range(ncb):
                r = cb - 4 * j
                lo = P * r if r > 0 else 0
                csl = bass.ts(cb, P)
                sc = ps_sc.tile([P, NP, TT], F32, tag="sc",
                                name=f"sc_{j}_{p}_{cb}")
                for h in range(NP):
                    hp = slice(DH * h, DH * h + DH)
                    nc.tensor.matmul(sc[:, h, lo:], kT[hp, p, csl],
                                     qT[hp, p, jsl][:, lo:],
                                     start=True, stop=True)
                pr = prp.tile([P, NP, TT], BF16, tag="pr",
                              name=f"pr_{j}_{p}_{cb}")
                if USE_STRIDED_EXP:
                    nc.scalar.activation(pr[:, :, lo:], sc[:, :, lo:],
                                         mybir.ActivationFunctionType.Exp,
                                         scale=0.125)
                else:
                    for h in range(NP):
                        nc.scalar.activation(
                            pr[:, h, lo:], sc[:, h, lo:],
                            mybir.ActivationFunctionType.Exp, scale=0.125)
                if r >= 0:
                    rsl = bass.ts(r, P)
                    for h in range(NP):
                        if USE_AFFSEL:
                            nc.gpsimd.affine_select(
                                out=pr[:, h, rsl], in_=pr[:, h, rsl],
                                compare_op=mybir.AluOpType.is_ge,
                                fill=0.0, base=0, pattern=[[1, P]],
                                channel_multiplier=-1)
                        else:
                            nc.vector.tensor_mul(pr[:, h, rsl],
                                                 pr[:, h, rsl],
                                                 mask_band[:])
                if pending is not None:
                    pcb, ppr, plo = pending
                    for h in range(NP):
                        nc.tensor.matmul(
                            pvh[h][:, plo:],
                            vn[:, pcb, 2 * p + h, :], ppr[:, h, plo:],
                            start=(pcb == 0), stop=False)
                pending = (cb, pr, lo)
                if j == 0 and p == 0 and cb == 0:
                    drain(2)   # V closures must emit before the first PV
                drain_step()
            pcb, ppr, plo = pending
            for h in range(NP):
                nc.tensor.matmul(pvh[h][:, plo:],
                                 vn[:, pcb, 2 * p + h, :], ppr[:, h, plo:],
                                 start=(pcb == 0), stop=True)

            # free the PV banks: unnormalized copy + denominator recip
            dsb = stage.tile([1, NP, TT], F32, tag=f"dsb{p}",
                             name=f"dsb_{j}_{p}")
            for h in range(NP):
                nc.vector.tensor_copy(attnT[bass.ts(h, DH), p, jsl],
                                      pvh[h][0:DH, :])
                nc.vector.tensor_copy(dsb[0:1, h, :],
                                      pvh[h][DH:DH + 1, :])
            if USE_FAST_RECIP:
                nc.vector.reciprocal_approx_fast(out=dinv[:], in_=dsb[:])
            else:
                nc.vector.reciprocal(dinv[:], dsb[:])

            # normalize tail as a deferred closure: emitted a few chunk
            # iterations into the NEXT sweep, so the Pool-queue broadcasts
            # never sit at the queue head waiting on the reciprocal chain
            # (head-of-line blocking the next sweep's affine_selects).
            # partition_broadcast does not honor a non-zero output base
            # partition on HW (head1 goes via a base-0 temp + copy); DVE
            # TensorTensor needs equal input base partitions in SBUF.
            def norm_tail(j=j, p=p, jsl=jsl, dinv=dinv):
                rbf = stage.tile([P, TT], F32, tag="rbf", name=f"rbf_{j}_{p}")
                nc.gpsimd.partition_broadcast(rbf[0:DH, :], dinv[0:1, 0, :])
                rbt = stage.tile([DH, TT], F32, tag="rbt", name=f"rbt_{j}_{p}")
                nc.gpsimd.partition_broadcast(rbt[:], dinv[0:1, 1, :])
                nc.vector.tensor_copy(rbf[DH:P, :], rbt[:])
                nc.vector.tensor_mul(attnT[:, p, jsl], attnT[:, p, jsl],
                                     rbf[:])
            work.appendleft(norm_tail)

        # tile 0: pair-0 q/k projections immediately (all attn(0) pair 0
        # needs); pair-1 + V closures drain at the first chunk so scoring
        # starts as soon as possible. xt1 is staged up front too (its DMA
        # shares the startup window), proj(1) drains during attn(0).
        qk0 = make_proj(0, xt0)
        for f in qk0[:4]:
            f()
        work.extend(qk0[4:])
        xt1 = xpool.tile([P, KD, TT], BF16, tag="xt", name="xt_1")
        for c in range(KD):
            nc.gpsimd.dma_start(xt1[:, c, :], xT[c, 1])
        work.extend(make_proj(1, xt1))

        for j in range(NT):
            ncb = 4 * (j + 1)
            # stage + enqueue tile j+1 at phase start (xt1 staged upfront);
            # the ration spreads its closures over BOTH pair sweeps, with a
            # few skipped iterations up front so drained matmuls never wait
            # on the just-issued xt DMA in the in-order PE queue.
            if 1 <= j and j + 1 < NT:
                xt = xpool.tile([P, KD, TT], BF16, tag="xt", name=f"xt_{j+1}")
                for c in range(KD):
                    nc.gpsimd.dma_start(xt[:, c, :], xT[c, j + 1])
                work.extend(make_proj(j + 1, xt))

            # attn(0) pair 0 runs while xt1/wv still stream: only the V
            # closures (explicit drain) go in; everything else would block
            # the in-order PE queue behind unarrived DMAs. Later phases
            # spread the drains over both sweeps, skipping a few iterations
            # so drained matmuls never wait on the just-issued xt DMA.
            set_ration(2 * ncb if j > 0 else 10**9, skip=4 if j > 0 else 0)
            pair_sweep(j, 0)
            if j == 0:
                set_ration(ncb)
            pair_sweep(j, 1)
            drain(len(work))   # proj(j+1) fully emitted before attn(j+1)
            work.extend(make_wo(j))
        drain(len(work))


_NC_CACHE = None


def _get_nc():
    global _NC_CACHE
    if _NC_CACHE is None:
        nc = bacc.Bacc("TRN2", target_bir_lowering=False, debug=False,
                       num_devices=N_CORES)
        with tile.TileContext(nc) as tc:
            _body(tc)
        nc.compile()
        _NC_CACHE = nc
    return _NC_CACHE


def _in_maps(x, W_Q, W_K, W_V, W_O):
    import ml_dtypes
    bf = ml_dtypes.bfloat16
    x32 = np.asarray(x, dtype=np.float32)
    W_Q = np.asarray(W_Q, dtype=np.float32)
    W_K = np.asarray(W_K, dtype=np.float32)
    W_V = np.asarray(W_V, dtype=np.float32)
    W_O = np.asarray(W_O, dtype=np.float32)
    # chunk-major layouts so every device DMA is a contiguous block
    xTb = [np.ascontiguousarray(
        x32[b].T.reshape(KD, P, NT, TT).transpose(0, 2, 1, 3)).astype(bf)
        for b in range(B)]
    maps = []
    for core in range(N_CORES):
        hq, b = core // 2, core % 2
        sl = slice(HD * hq, HD * hq + HD)
        maps.append({
            "xT": xTb[b],
            "wq": np.ascontiguousarray(W_Q[:, sl].reshape(KD, P, HD)).astype(bf),
            "wk": np.ascontiguousarray(W_K[:, sl].reshape(KD, P, HD)).astype(bf),
            "wv": np.ascontiguousarray(W_V[:, sl].reshape(KD, P, HD)).astype(bf),
            "wo": np.ascontiguousarray(
                W_O[sl, :].reshape(NP, P, D).transpose(1, 0, 2)).astype(bf),
        })
    return maps


def _gather(results):
    acc = np.zeros([B, D, S], np.float32)
    for core, r in enumerate(results):
        hq, b = core // 2, core % 2
        acc[b] += r["outT"].astype(np.float32).transpose(
            0, 2, 1, 3).reshape(D, S)
    return np.ascontiguousarray(acc.transpose(0, 2, 1))


def kernel(x, W_Q, W_K, W_V, W_O):
    nc = _get_nc()
    res = run_bass_kernel_spmd(nc, _in_maps(x, W_Q, W_K, W_V, W_O),
                               core_ids=list(range(N_CORES)))
    return _gather(res.results)


def kernel_profiled(x, W_Q, W_K, W_V, W_O):
    """Like kernel() but with NTFF tracing; returns (output, exec_time_ns)."""
    nc = _get_nc()
    res = run_bass_kernel_spmd(nc, _in_maps(x, W_Q, W_K, W_V, W_O),
                               core_ids=list(range(N_CORES)), trace=True)
    return _gather(res.results), res.exec_time_ns


# revision 4
# speedup vs baseline: 1.0464x; 1.0278x over previous
"""Causal multi-head attention on 8 Trainium2 NeuronCores.

Problem: x[2,2048,1024] @ W_Q/K/V[1024,1024] -> 16-head causal attention
(d_head=64) -> @ W_O[1024,1024].

Sharding: core c = (hq, b) with hq = c//2, b = c%2. Core owns heads
4hq..4hq+3 (columns [256hq:256hq+256) of W_Q/K/V, rows of W_O) and batch b.
Each core computes a partial output [1024, 2048] (transposed, bf16); the
host sums the 4 head-quad partials per batch and un-transposes.

Design (per core, bf16 operands, fp32 PSUM):
  - Fused pipeline over 4 token tiles of 512: project tile t, causal
    attention for q-tile j=t (its K/V chunks are all ready), W_O for tile
    j - with projection/W_O matmuls of neighboring tiles rationed into
    the in-order PE queue between attention chunks so the PE stays busy
    (HAM stays warm) while the scalar engine (exp) paces the steady state.
  - Q/K projected transposed ([dims, tokens]); V projected directly in
    natural [token, dim] layout (lhsT = x chunk) - no PE transposes.
  - Heads packed in pairs on partitions 0-63 / 64-127; the two score
    matmuls of a pair run on disjoint PE row groups (concurrent on HW).
  - One exp per (pair, chunk) over [128, 2, 512] straddling two PSUM
    banks; causal band masking via gpsimd affine_select on the probs.
  - PV accumulates [96, 512] per head (64 dims + ones column + pad; the
    ones column yields the softmax denominator for free). Reciprocal via
    the fast custom DVE op on base-partition-0 staging tiles; the
    normalize broadcasts+multiply are deferred into the next sweep so
    they never head-of-line-block the Pool queue.
  - All DRAM layouts chunk-major so every DMA moves contiguous blocks.
PSUM: scores 2x[128,2,512] (4 banks) + PV [96,2,512] (2) + proj/WO ring
2x[128,512] (2) = 8 banks.

Measured: 377us (fp32r phase-serial baseline) -> ~181us, rel err 3.9e-3.
"""

from collections import deque

import numpy as np

import concourse.bass as bass
import concourse.tile as tile
from concourse import bacc, mybir
from concourse.bass_utils import run_bass_kernel_spmd

F32 = mybir.dt.float32
BF16 = mybir.dt.bfloat16

N_CORES = 8
P = 128

# HW-bisection flags (CoreSim passes all combinations; HW may not)
USE_STRIDED_EXP = True    # one exp over [128, 2, 512] vs per-head 2D exps
USE_AFFSEL = True         # causal mask via gpsimd.affine_select on probs
USE_FAST_RECIP = True     # reciprocal_approx_fast direct from PSUM
XPOOL_BUFS = 4            # x-tile ring depth; 2 races on HW (DMA vs PE reads)
D = 1024          # d_model
B = 2             # batch
S = 2048          # seq len per core (one batch)
TT = 512          # token tile (free dim of matmuls)
NT = S // TT      # 4 token tiles
KD = D // P       # 8 contraction chunks for projections
NCH = S // P      # 16 k-chunks
HD = 256          # head dims per core (4 heads x 64)
DH = 64           # head dim
NP = 2            # head pairs per core
VW = 96           # V block width: 64 dims + ones col + pad to a 32-multiple


def _body(tc):
    nc = tc.nc
    # all DRAM layouts are chunk-major so every DMA moves a contiguous
    # [128, *] block (strided descriptors halve DMA throughput)
    xT = nc.dram_tensor("xT", [KD, NT, P, TT], BF16, kind="ExternalInput").ap()
    wq = nc.dram_tensor("wq", [KD, P, HD], BF16, kind="ExternalInput").ap()
    wk = nc.dram_tensor("wk", [KD, P, HD], BF16, kind="ExternalInput").ap()
    wv = nc.dram_tensor("wv", [KD, P, HD], BF16, kind="ExternalInput").ap()
    wo = nc.dram_tensor("wo", [P, NP, D], BF16, kind="ExternalInput").ap()
    outT = nc.dram_tensor("outT", [KD, NT, P, TT], BF16,
                          kind="ExternalOutput").ap()

    import contextlib
    with contextlib.ExitStack() as ctx:
        wpool = ctx.enter_context(tc.tile_pool(name="wpool", bufs=1))
        xpool = ctx.enter_context(tc.tile_pool(name="xpool", bufs=XPOOL_BUFS))
        persist = ctx.enter_context(tc.tile_pool(name="persist", bufs=1))
        prp = ctx.enter_context(tc.tile_pool(name="probs", bufs=6))
        stage = ctx.enter_context(tc.tile_pool(name="stage", bufs=3))
        obp = ctx.enter_context(tc.tile_pool(name="obp", bufs=3))
        ps_sc = ctx.enter_context(tc.tile_pool(name="ps_sc", bufs=2, space="PSUM"))
        ps_pv = ctx.enter_context(tc.tile_pool(name="ps_pv", bufs=1, space="PSUM"))
        ps_pw = ctx.enter_context(tc.tile_pool(name="ps_pw", bufs=2, space="PSUM"))

        # mask_band[k, q] = 1.0 if q >= k else 0.0 (only for USE_AFFSEL=False)
        mask_band = None
        if not USE_AFFSEL:
            mask_band = wpool.tile([P, P], F32)
            nc.any.memset(mask_band[:], 1.0)
            nc.gpsimd.affine_select(
                out=mask_band[:], in_=mask_band[:],
                compare_op=mybir.AluOpType.is_ge,
                fill=0.0, base=0, pattern=[[1, P]],
                channel_multiplier=-1)

        # --- PE warmup: dummy matmuls keep the HAM clock-gate warm while
        # the initial DMAs stream in (no data deps, results discarded)
        dum = wpool.tile([P, TT], BF16)
        nc.vector.memset(dum[:], 0.0)
        dps = ps_pw.tile([P, TT], F32, tag="pw", name="warmup")
        for _ in range(14):
            nc.tensor.matmul(dps[:], dum[:, 0:P], dum[:],
                             start=True, stop=True)

        # --- weights (per-chunk DMAs so the first matmuls start early) ----
        xt0 = xpool.tile([P, KD, TT], BF16, tag="xt", name="xt_0")
        wq_sb = wpool.tile([P, KD, HD], BF16)
        wk_sb = wpool.tile([P, KD, HD], BF16)
        wv_sb = wpool.tile([P, KD, HD], BF16)
        for c in range(KD):
            nc.gpsimd.dma_start(xt0[:, c, :], xT[c, 0])
            nc.sync.dma_start(wq_sb[:, c, :], wq[c])
            nc.sync.dma_start(wk_sb[:, c, :], wk[c])
        for c in range(KD):   # wv only feeds the (deferred) V projection
            nc.sync.dma_start(wv_sb[:, c, :], wv[c])
        wo_sb = wpool.tile([P, NP, D], BF16)
        nc.sync.dma_start(wo_sb[:], wo)

        # --- persistent activations ---------------------------------------
        qT = persist.tile([P, NP, S], BF16)    # pair p: head 2p on part 0-63
        kT = persist.tile([P, NP, S], BF16)
        vn = persist.tile([P, NCH, 4, VW], BF16)  # [tok, chunk, head, d|1|pad]
        attnT = persist.tile([P, NP, S], BF16)
        # ones columns via memset (NOT activation(in*0+1): uninitialized SBUF
        # may hold NaN and NaN*0 = NaN on hardware); zero the pad columns
        for l in range(4):
            nc.vector.memset(vn[:, :, l, DH], 1.0)
            nc.vector.memset(vn[:, :, l, DH + 1:VW], 0.0)

        # deferred work: closures emitting ~1-2us of PE work each, drained
        # between attention chunk iterations to keep the PE queue dense.
        # Rationed so the queue lasts the whole attention phase (the PE
        # would otherwise run dry near pair transitions and HAM-rethrottle).
        work = deque()
        ration = [0.0, 0.0]   # step, accumulator

        def set_ration(iters, skip=0):
            ration[0] = len(work) / max(iters - skip, 1)
            ration[1] = -skip * ration[0]

        def drain_step():
            ration[1] += ration[0]
            while ration[1] >= 1.0 and work:
                work.popleft()()
                ration[1] -= 1.0

        def drain(n):
            for _ in range(min(n, len(work))):
                work.popleft()()

        def make_proj(t, xt):
            """Projection closures for token tile t (uses staged xt)."""
            ops = []
            tsl = bass.ts(t, TT)

            def qk_proj(wsb, dstT, p, t=t, xt=xt, tsl=tsl):
                def f():
                    ps = ps_pw.tile([P, TT], F32, tag="pw",
                                    name=f"pj_{wsb is wk_sb}_{t}_{p}")
                    for c in range(KD):
                        nc.tensor.matmul(ps[:], wsb[:, c, bass.ts(p, P)],
                                         xt[:, c, :],
                                         start=(c == 0), stop=(c == KD - 1))
                    nc.vector.tensor_copy(dstT[:, p, tsl], ps[:])
                return f

            def v_proj(tc0, t=t, xt=xt):
                def f():
                    ps = ps_pw.tile([P, 2, HD], F32, tag="pw",
                                    name=f"pv_{t}_{tc0}")
                    for i in range(2):
                        tci = tc0 + i
                        for c in range(KD):
                            nc.tensor.matmul(
                                ps[:, i, :],
                                xt[:, c, bass.ts(tci, P)],
                                wv_sb[:, c, :],
                                start=(c == 0), stop=(c == KD - 1))
                    for i in range(2):
                        ch = t * 4 + tc0 + i
                        nc.vector.tensor_copy(
                            vn[:, ch, :, 0:DH],
                            ps[:, i, :].rearrange("p (h d) -> p h d", h=4))
                return f

            for p in range(NP):
                ops.append(qk_proj(wq_sb, qT, p))
                ops.append(qk_proj(wk_sb, kT, p))
            return ops, [v_proj(0), v_proj(2)]

        def make_wo(j):
            """Output projection closures for q-tile j (reads attnT)."""
            ops = []
            jsl = bass.ts(j, TT)
            # the last tile's W_O runs after attention ends: borrow the
            # score-PSUM ring (free by then) for 2x the groups in flight
            wpool_ps = ps_sc if j == NT - 1 else ps_pw
            wtag = "sc" if j == NT - 1 else "pw"
            for f_ in range(KD):
                def g(f_=f_, jsl=jsl, j=j):
                    ps = wpool_ps.tile([P, TT], F32, tag=wtag,
                                       name=f"wo_{j}_{f_}")
                    for a in range(NP):
                        nc.tensor.matmul(ps[:], wo_sb[:, a, bass.ts(f_, P)],
                                         attnT[:, a, jsl],
                                         start=(a == 0), stop=(a == NP - 1))
                    ob = obp.tile([P, TT], BF16, tag="ob", name=f"ob_{j}_{f_}")
                    if j == NT - 1 and f_ % 2:
                        # tail: ACT is idle after the last exp — share the
                        # PSUM drain copies between the two engines
                        nc.scalar.activation(
                            ob[:], ps[:], mybir.ActivationFunctionType.Copy)
                    else:
                        nc.vector.tensor_copy(ob[:], ps[:])
                    nc.sync.dma_start(outT[f_, j], ob[:])
                ops.append(g)
            return ops

        def pair_sweep(j, p):
            """Scores+exp+mask+PV chunk loop for (q-tile j, head pair p),
            followed by the PSUM drain copies, reciprocal, and per-pair
            normalize of attnT."""
            jsl = bass.ts(j, TT)
            ncb = 4 * (j + 1)
            dinv = stage.tile([1, NP, TT], F32, tag=f"dinv{p}",
                              name=f"dinv_{j}_{p}")
            pv = ps_pv.tile([VW, NP, TT], F32, tag="pv", name=f"pv_{j}_{p}")
            pvh = [pv[:, h, :] for h in range(NP)]
            pending = None
            for cb in range(ncb):
                r = cb - 4 * j
                lo = P * r if r > 0 else 0
                csl = bass.ts(cb, P)
                sc = ps_sc.tile([P, NP, TT], F32, tag="sc",
                                name=f"sc_{j}_{p}_{cb}")
                for h in range(NP):
                    hp = slice(DH * h, DH * h + DH)
                    nc.tensor.matmul(sc[:, h, lo:], kT[hp, p, csl],
                                     qT[hp, p, jsl][:, lo:],
                                     start=True, stop=True)
                pr = prp.tile([P, NP, TT], BF16, tag="pr",
                              name=f"pr_{j}_{p}_{cb}")
                if USE_STRIDED_EXP:
                    nc.scalar.activation(pr[:, :, lo:], sc[:, :, lo:],
                                         mybir.ActivationFunctionType.Exp,
                                         scale=0.125)
                else:
                    for h in range(NP):
                        nc.scalar.activation(
                            pr[:, h, lo:], sc[:, h, lo:],
                            mybir.ActivationFunctionType.Exp, scale=0.125)
                if r >= 0:
                    rsl = bass.ts(r, P)
                    for h in range(NP):
                        if USE_AFFSEL:
                            nc.gpsimd.affine_select(
                                out=pr[:, h, rsl], in_=pr[:, h, rsl],
                                compare_op=mybir.AluOpType.is_ge,
                                fill=0.0, base=0, pattern=[[1, P]],
                                channel_multiplier=-1)
                        else:
                            nc.vector.tensor_mul(pr[:, h, rsl],
                                                 pr[:, h, rsl],
                                                 mask_band[:])
                if pending is not None:
                    pcb, ppr, plo = pending
                    for h in range(NP):
                        nc.tensor.matmul(
                            pvh[h][:, plo:],
                            vn[:, pcb, 2 * p + h, :], ppr[:, h, plo:],
                            start=(pcb == 0), stop=False)
                pending = (cb, pr, lo)
                # V-projection closures for this tile drain inside pair 0:
                # their matmuls are absorbed by the per-chunk ACT-vs-PE
                # slack instead of stalling the exp stream at the phase
                # boundary. vn chunk c is only read at pv-emission c+1, so
                # j=0 needs them at cb0; later tiles any time before cb=4j.
                if p == 0 and vwork.get(j):
                    if cb in ((0, 1) if j == 0 else (2, 3)):
                        vwork[j].pop(0)()
                drain_step()
            pcb, ppr, plo = pending
            for h in range(NP):
                nc.tensor.matmul(pvh[h][:, plo:],
                                 vn[:, pcb, 2 * p + h, :], ppr[:, h, plo:],
                                 start=(pcb == 0), stop=True)

            # free the PV banks: unnormalized copy + denominator recip
            dsb = stage.tile([1, NP, TT], F32, tag=f"dsb{p}",
                             name=f"dsb_{j}_{p}")
            for h in range(NP):
                nc.vector.tensor_copy(attnT[bass.ts(h, DH), p, jsl],
                                      pvh[h][0:DH, :])
                nc.vector.tensor_copy(dsb[0:1, h, :],
                                      pvh[h][DH:DH + 1, :])
            if USE_FAST_RECIP:
                nc.vector.reciprocal_approx_fast(out=dinv[:], in_=dsb[:])
            else:
                nc.vector.reciprocal(dinv[:], dsb[:])

            # normalize tail as a deferred closure: emitted a few chunk
            # iterations into the NEXT sweep, so the Pool-queue broadcasts
            # never sit at the queue head waiting on the reciprocal chain
            # (head-of-line blocking the next sweep's affine_selects).
            # partition_broadcast does not honor a non-zero output base
            # partition on HW (head1 goes via a base-0 temp + copy); DVE
            # TensorTensor needs equal input base partitions in SBUF.
            def norm_tail(j=j, p=p, jsl=jsl, dinv=dinv):
                rbf = stage.tile([P, TT], F32, tag="rbf", name=f"rbf_{j}_{p}")
                nc.gpsimd.partition_broadcast(rbf[0:DH, :], dinv[0:1, 0, :])
                rbt = stage.tile([DH, TT], F32, tag="rbt", name=f"rbt_{j}_{p}")
                nc.gpsimd.partition_broadcast(rbt[:], dinv[0:1, 1, :])
                nc.vector.tensor_copy(rbf[DH:P, :], rbt[:])
                nc.vector.tensor_mul(attnT[:, p, jsl], attnT[:, p, jsl],
                                     rbf[:])
            work.appendleft(norm_tail)

        # tile 0: pair-0 q/k projections immediately (all attn(0) pair 0
        # needs); pair-1 + V closures drain at the first chunk so scoring
        # starts as soon as possible. xt1 is staged up front too (its DMA
        # shares the startup window), proj(1) drains during attn(0).
        vwork = {}
        qk0, vwork[0] = make_proj(0, xt0)
        for f in qk0:
            f()
        xt1 = xpool.tile([P, KD, TT], BF16, tag="xt", name="xt_1")
        for c in range(KD):
            nc.gpsimd.dma_start(xt1[:, c, :], xT[c, 1])
        qk1, vwork[1] = make_proj(1, xt1)
        work.extend(qk1)

        for j in range(NT):
            ncb = 4 * (j + 1)
            # stage + enqueue tile j+1 at phase start (xt1 staged upfront);
            # the ration spreads its closures over BOTH pair sweeps, with a
            # few skipped iterations up front so drained matmuls never wait
            # on the just-issued xt DMA in the in-order PE queue.
            if 1 <= j and j + 1 < NT:
                xt = xpool.tile([P, KD, TT], BF16, tag="xt", name=f"xt_{j+1}")
                for c in range(KD):
                    nc.gpsimd.dma_start(xt[:, c, :], xT[c, j + 1])
                nxtqk, vwork[j + 1] = make_proj(j + 1, xt)
                # q/k closures gate attn(j+1)'s start — drain them FIRST
                # (ahead of leftover W_O work); V drains inside pair 0
                work.extendleft(reversed(nxtqk))

            # attn(0) pair 0 runs while xt1/wv still stream: only the V
            # closures (explicit drain) go in; everything else would block
            # the in-order PE queue behind unarrived DMAs. Later phases
            # spread the drains over both sweeps, skipping a few iterations
            # so drained matmuls never wait on the just-issued xt DMA.
            set_ration(2 * ncb if j > 0 else 10**9, skip=4 if j > 0 else 0)
            pair_sweep(j, 0)
            if j == 0:
                set_ration(ncb)
            pair_sweep(j, 1)
            drain(len(work))   # proj(j+1) fully emitted before attn(j+1)
            work.extend(make_wo(j))
        drain(len(work))


_NC_CACHE = None


def _get_nc():
    global _NC_CACHE
    if _NC_CACHE is None:
        nc = bacc.Bacc("TRN2", target_bir_lowering=False, debug=False,
                       num_devices=N_CORES)
        with tile.TileContext(nc) as tc:
            _body(tc)
        nc.compile()
        _NC_CACHE = nc
    return _NC_CACHE


def _in_maps(x, W_Q, W_K, W_V, W_O):
    import ml_dtypes
    bf = ml_dtypes.bfloat16
    x32 = np.asarray(x, dtype=np.float32)
    W_Q = np.asarray(W_Q, dtype=np.float32)
    W_K = np.asarray(W_K, dtype=np.float32)
    W_V = np.asarray(W_V, dtype=np.float32)
    W_O = np.asarray(W_O, dtype=np.float32)
    # chunk-major layouts so every device DMA is a contiguous block
    xTb = [np.ascontiguousarray(
        x32[b].T.reshape(KD, P, NT, TT).transpose(0, 2, 1, 3)).astype(bf)
        for b in range(B)]
    maps = []
    for core in range(N_CORES):
        hq, b = core // 2, core % 2
        sl = slice(HD * hq, HD * hq + HD)
        maps.append({
            "xT": xTb[b],
            "wq": np.ascontiguousarray(W_Q[:, sl].reshape(KD, P, HD)).astype(bf),
            "wk": np.ascontiguousarray(W_K[:, sl].reshape(KD, P, HD)).astype(bf),
            "wv": np.ascontiguousarray(W_V[:, sl].reshape(KD, P, HD)).astype(bf),
            "wo": np.ascontiguousarray(
                W_O[sl, :].reshape(NP, P, D).transpose(1, 0, 2)).astype(bf),
        })
    return maps


def _gather(results):
    acc = np.zeros([B, D, S], np.float32)
    for core, r in enumerate(results):
        hq, b = core // 2, core % 2
        acc[b] += r["outT"].astype(np.float32).transpose(
            0, 2, 1, 3).reshape(D, S)
    return np.ascontiguousarray(acc.transpose(0, 2, 1))


def kernel(x, W_Q, W_K, W_V, W_O):
    nc = _get_nc()
    res = run_bass_kernel_spmd(nc, _in_maps(x, W_Q, W_K, W_V, W_O),
                               core_ids=list(range(N_CORES)))
    return _gather(res.results)


def kernel_profiled(x, W_Q, W_K, W_V, W_O):
    """Like kernel() but with NTFF tracing; returns (output, exec_time_ns)."""
    nc = _get_nc()
    res = run_bass_kernel_spmd(nc, _in_maps(x, W_Q, W_K, W_V, W_O),
                               core_ids=list(range(N_CORES)), trace=True)
    return _gather(res.results), res.exec_time_ns


# revision 5
# speedup vs baseline: 1.0580x; 1.0111x over previous
"""Causal multi-head attention on 8 Trainium2 NeuronCores.

Problem: x[2,2048,1024] @ W_Q/K/V[1024,1024] -> 16-head causal attention
(d_head=64) -> @ W_O[1024,1024].

Sharding: core c = (hq, b) with hq = c//2, b = c%2. Core owns heads
4hq..4hq+3 (columns [256hq:256hq+256) of W_Q/K/V, rows of W_O) and batch b.
Each core computes a partial output [1024, 2048] (transposed, bf16); the
host sums the 4 head-quad partials per batch and un-transposes.

Design (per core, bf16 operands, fp32 PSUM):
  - Fused pipeline over 4 token tiles of 512: project tile t, causal
    attention for q-tile j=t (its K/V chunks are all ready), W_O for tile
    j - with projection/W_O matmuls of neighboring tiles rationed into
    the in-order PE queue between attention chunks so the PE stays busy
    (HAM stays warm) while the scalar engine (exp) paces the steady state.
  - Q/K projected transposed ([dims, tokens]); V projected directly in
    natural [token, dim] layout (lhsT = x chunk) - no PE transposes.
  - Heads packed in pairs on partitions 0-63 / 64-127; the two score
    matmuls of a pair run on disjoint PE row groups (concurrent on HW).
  - One exp per (pair, chunk) over [128, 2, 512] straddling two PSUM
    banks; causal band masking via gpsimd affine_select on the probs.
  - PV accumulates [96, 512] per head (64 dims + ones column + pad; the
    ones column yields the softmax denominator for free). Reciprocal via
    the fast custom DVE op on base-partition-0 staging tiles; the
    normalize broadcasts+multiply are deferred into the next sweep so
    they never head-of-line-block the Pool queue.
  - All DRAM layouts chunk-major so every DMA moves contiguous blocks.
PSUM: scores 2x[128,2,512] (4 banks) + PV [96,2,512] (2) + proj/WO ring
2x[128,512] (2) = 8 banks.

Measured: 377us (fp32r phase-serial baseline) -> ~181us, rel err 3.9e-3.
"""

from collections import deque

import numpy as np

import concourse.bass as bass
import concourse.tile as tile
from concourse import bacc, mybir
from concourse.bass_utils import run_bass_kernel_spmd

F32 = mybir.dt.float32
BF16 = mybir.dt.bfloat16

N_CORES = 8
P = 128

# HW-bisection flags (CoreSim passes all combinations; HW may not)
USE_STRIDED_EXP = True    # one exp over [128, 2, 512] vs per-head 2D exps
USE_AFFSEL = True         # causal mask via gpsimd.affine_select on probs
USE_FAST_RECIP = True     # reciprocal_approx_fast direct from PSUM
XPOOL_BUFS = 4            # x-tile ring depth; 2 races on HW (DMA vs PE reads)
D = 1024          # d_model
B = 2             # batch
S = 2048          # seq len per core (one batch)
TT = 512          # token tile (free dim of matmuls)
NT = S // TT      # 4 token tiles
KD = D // P       # 8 contraction chunks for projections
NCH = S // P      # 16 k-chunks
HD = 256          # head dims per core (4 heads x 64)
DH = 64           # head dim
NP = 2            # head pairs per core
VW = 96           # V block width: 64 dims + ones col + pad to a 32-multiple


def _body(tc):
    nc = tc.nc
    # all DRAM layouts are chunk-major so every DMA moves a contiguous
    # [128, *] block (strided descriptors halve DMA throughput)
    xT = nc.dram_tensor("xT", [KD, NT, P, TT], BF16, kind="ExternalInput").ap()
    wq = nc.dram_tensor("wq", [KD, P, HD], BF16, kind="ExternalInput").ap()
    wk = nc.dram_tensor("wk", [KD, P, HD], BF16, kind="ExternalInput").ap()
    wv = nc.dram_tensor("wv", [KD, P, HD], BF16, kind="ExternalInput").ap()
    wo = nc.dram_tensor("wo", [P, NP, D], BF16, kind="ExternalInput").ap()
    outT = nc.dram_tensor("outT", [KD, NT, P, TT], BF16,
                          kind="ExternalOutput").ap()

    import contextlib
    with contextlib.ExitStack() as ctx:
        wpool = ctx.enter_context(tc.tile_pool(name="wpool", bufs=1))
        xpool = ctx.enter_context(tc.tile_pool(name="xpool", bufs=XPOOL_BUFS))
        persist = ctx.enter_context(tc.tile_pool(name="persist", bufs=1))
        prp = ctx.enter_context(tc.tile_pool(name="probs", bufs=8))
        stage = ctx.enter_context(tc.tile_pool(name="stage", bufs=3))
        obp = ctx.enter_context(tc.tile_pool(name="obp", bufs=3))
        ps_sc = ctx.enter_context(tc.tile_pool(name="ps_sc", bufs=2, space="PSUM"))
        ps_pv = ctx.enter_context(tc.tile_pool(name="ps_pv", bufs=1, space="PSUM"))
        ps_pw = ctx.enter_context(tc.tile_pool(name="ps_pw", bufs=2, space="PSUM"))

        # mask_band[k, q] = 1.0 if q >= k else 0.0 (only for USE_AFFSEL=False)
        mask_band = None
        if not USE_AFFSEL:
            mask_band = wpool.tile([P, P], F32)
            nc.any.memset(mask_band[:], 1.0)
            nc.gpsimd.affine_select(
                out=mask_band[:], in_=mask_band[:],
                compare_op=mybir.AluOpType.is_ge,
                fill=0.0, base=0, pattern=[[1, P]],
                channel_multiplier=-1)

        # --- PE warmup: dummy matmuls keep the HAM clock-gate warm while
        # the initial DMAs stream in (no data deps, results discarded)
        dum = wpool.tile([P, TT], BF16)
        nc.vector.memset(dum[:], 0.0)
        dps = ps_pw.tile([P, TT], F32, tag="pw", name="warmup")
        for _ in range(14):
            nc.tensor.matmul(dps[:], dum[:, 0:P], dum[:],
                             start=True, stop=True)

        # --- weights (per-chunk DMAs so the first matmuls start early) ----
        xt0 = xpool.tile([P, KD, TT], BF16, tag="xt", name="xt_0")
        wq_sb = wpool.tile([P, KD, HD], BF16)
        wk_sb = wpool.tile([P, KD, HD], BF16)
        wv_sb = wpool.tile([P, KD, HD], BF16)
        for c in range(KD):
            nc.gpsimd.dma_start(xt0[:, c, :], xT[c, 0])
            nc.sync.dma_start(wq_sb[:, c, :], wq[c])
            nc.sync.dma_start(wk_sb[:, c, :], wk[c])
        for c in range(KD):   # wv only feeds the (deferred) V projection
            nc.sync.dma_start(wv_sb[:, c, :], wv[c])
        wo_sb = wpool.tile([P, NP, D], BF16)
        nc.sync.dma_start(wo_sb[:], wo)

        # --- persistent activations ---------------------------------------
        qT = persist.tile([P, NP, S], BF16)    # pair p: head 2p on part 0-63
        kT = persist.tile([P, NP, S], BF16)
        vn = persist.tile([P, NCH, 4, VW], BF16)  # [tok, chunk, head, d|1|pad]
        attnT = persist.tile([P, NP, S], BF16)
        # ones columns via memset (NOT activation(in*0+1): uninitialized SBUF
        # may hold NaN and NaN*0 = NaN on hardware); zero the pad columns
        for l in range(4):
            nc.vector.memset(vn[:, :, l, DH], 1.0)
            nc.vector.memset(vn[:, :, l, DH + 1:VW], 0.0)

        # deferred work: closures emitting ~1-2us of PE work each, drained
        # between attention chunk iterations to keep the PE queue dense.
        # Rationed so the queue lasts the whole attention phase (the PE
        # would otherwise run dry near pair transitions and HAM-rethrottle).
        work = deque()
        ration = [0.0, 0.0]   # step, accumulator

        def set_ration(iters, skip=0):
            ration[0] = len(work) / max(iters - skip, 1)
            ration[1] = -skip * ration[0]

        def drain_step():
            ration[1] += ration[0]
            while ration[1] >= 1.0 and work:
                work.popleft()()
                ration[1] -= 1.0

        def drain(n):
            for _ in range(min(n, len(work))):
                work.popleft()()

        def make_proj(t, xt):
            """Projection closures for token tile t (uses staged xt)."""
            ops = []
            tsl = bass.ts(t, TT)

            def qk_proj(wsb, dstT, p, t=t, xt=xt, tsl=tsl):
                def f():
                    ps = ps_pw.tile([P, TT], F32, tag="pw",
                                    name=f"pj_{wsb is wk_sb}_{t}_{p}")
                    for c in range(KD):
                        nc.tensor.matmul(ps[:], wsb[:, c, bass.ts(p, P)],
                                         xt[:, c, :],
                                         start=(c == 0), stop=(c == KD - 1))
                    nc.vector.tensor_copy(dstT[:, p, tsl], ps[:])
                return f

            def v_proj(tc0, t=t, xt=xt):
                def f():
                    ps = ps_pw.tile([P, 2, HD], F32, tag="pw",
                                    name=f"pv_{t}_{tc0}")
                    for i in range(2):
                        tci = tc0 + i
                        for c in range(KD):
                            nc.tensor.matmul(
                                ps[:, i, :],
                                xt[:, c, bass.ts(tci, P)],
                                wv_sb[:, c, :],
                                start=(c == 0), stop=(c == KD - 1))
                    for i in range(2):
                        ch = t * 4 + tc0 + i
                        nc.vector.tensor_copy(
                            vn[:, ch, :, 0:DH],
                            ps[:, i, :].rearrange("p (h d) -> p h d", h=4))
                return f

            for p in range(NP):
                ops.append(qk_proj(wq_sb, qT, p))
                ops.append(qk_proj(wk_sb, kT, p))
            return ops, [v_proj(0), v_proj(2)]

        def make_wo(j):
            """Output projection closures for q-tile j (reads attnT)."""
            ops = []
            jsl = bass.ts(j, TT)
            # the last tile's W_O runs after attention ends: borrow the
            # score-PSUM ring (free by then) for 2x the groups in flight
            wpool_ps = ps_sc if j == NT - 1 else ps_pw
            wtag = "sc" if j == NT - 1 else "pw"
            for f_ in range(KD):
                def g(f_=f_, jsl=jsl, j=j):
                    ps = wpool_ps.tile([P, TT], F32, tag=wtag,
                                       name=f"wo_{j}_{f_}")
                    for a in range(NP):
                        nc.tensor.matmul(ps[:], wo_sb[:, a, bass.ts(f_, P)],
                                         attnT[:, a, jsl],
                                         start=(a == 0), stop=(a == NP - 1))
                    ob = obp.tile([P, TT], BF16, tag="ob", name=f"ob_{j}_{f_}")
                    if j == NT - 1 and f_ % 2:
                        # tail: ACT is idle after the last exp — share the
                        # PSUM drain copies between the two engines
                        nc.scalar.activation(
                            ob[:], ps[:], mybir.ActivationFunctionType.Copy)
                    else:
                        nc.vector.tensor_copy(ob[:], ps[:])
                    nc.sync.dma_start(outT[f_, j], ob[:])
                ops.append(g)
            return ops

        def pair_sweep(j, p):
            """Scores+exp+mask+PV chunk loop for (q-tile j, head pair p),
            followed by the PSUM drain copies, reciprocal, and per-pair
            normalize of attnT."""
            jsl = bass.ts(j, TT)
            ncb = 4 * (j + 1)
            dinv = stage.tile([1, NP, TT], F32, tag=f"dinv{p}",
                              name=f"dinv_{j}_{p}")
            pv = ps_pv.tile([VW, NP, TT], F32, tag="pv", name=f"pv_{j}_{p}")
            pvh = [pv[:, h, :] for h in range(NP)]
            pending = None
            for cb in range(ncb):
                r = cb - 4 * j
                lo = P * r if r > 0 else 0
                csl = bass.ts(cb, P)
                sc = ps_sc.tile([P, NP, TT], F32, tag="sc",
                                name=f"sc_{j}_{p}_{cb}")
                for h in range(NP):
                    hp = slice(DH * h, DH * h + DH)
                    nc.tensor.matmul(sc[:, h, lo:], kT[hp, p, csl],
                                     qT[hp, p, jsl][:, lo:],
                                     start=True, stop=True)
                pr = prp.tile([P, NP, TT], BF16, tag="pr",
                              name=f"pr_{j}_{p}_{cb}")
                if USE_STRIDED_EXP:
                    nc.scalar.activation(pr[:, :, lo:], sc[:, :, lo:],
                                         mybir.ActivationFunctionType.Exp,
                                         scale=0.125)
                else:
                    for h in range(NP):
                        nc.scalar.activation(
                            pr[:, h, lo:], sc[:, h, lo:],
                            mybir.ActivationFunctionType.Exp, scale=0.125)
                if r >= 0:
                    rsl = bass.ts(r, P)
                    for h in range(NP):
                        if USE_AFFSEL:
                            nc.gpsimd.affine_select(
                                out=pr[:, h, rsl], in_=pr[:, h, rsl],
                                compare_op=mybir.AluOpType.is_ge,
                                fill=0.0, base=0, pattern=[[1, P]],
                                channel_multiplier=-1)
                        else:
                            nc.vector.tensor_mul(pr[:, h, rsl],
                                                 pr[:, h, rsl],
                                                 mask_band[:])
                if pending is not None:
                    pcb, ppr, plo = pending
                    for h in range(NP):
                        nc.tensor.matmul(
                            pvh[h][:, plo:],
                            vn[:, pcb, 2 * p + h, :], ppr[:, h, plo:],
                            start=(pcb == 0), stop=False)
                pending = (cb, pr, lo)
                # V-projection closures for this tile drain inside pair 0:
                # their matmuls are absorbed by the per-chunk ACT-vs-PE
                # slack instead of stalling the exp stream at the phase
                # boundary. vn chunk c is only read at pv-emission c+1, so
                # j=0 needs them at cb0; later tiles any time before cb=4j.
                if p == 0 and vwork.get(j):
                    if cb in ((0, 1) if j == 0 else (2, 3)):
                        vwork[j].pop(0)()
                drain_step()
            pcb, ppr, plo = pending
            for h in range(NP):
                nc.tensor.matmul(pvh[h][:, plo:],
                                 vn[:, pcb, 2 * p + h, :], ppr[:, h, plo:],
                                 start=(pcb == 0), stop=True)

            # free the PV banks: unnormalized copy + denominator recip.
            # For the very last pair, ACT is idle (no exps left) — let it
            # take the attnT drain copies so the DVE reciprocal chain (and
            # with it W_O) starts ~2.6us earlier.
            tail = (j == NT - 1 and p == NP - 1)
            dsb = stage.tile([1, NP, TT], F32, tag=f"dsb{p}",
                             name=f"dsb_{j}_{p}")
            for h in range(NP):
                if tail:
                    nc.scalar.activation(attnT[bass.ts(h, DH), p, jsl],
                                         pvh[h][0:DH, :],
                                         mybir.ActivationFunctionType.Copy)
                else:
                    nc.vector.tensor_copy(attnT[bass.ts(h, DH), p, jsl],
                                          pvh[h][0:DH, :])
                nc.vector.tensor_copy(dsb[0:1, h, :],
                                      pvh[h][DH:DH + 1, :])
            if USE_FAST_RECIP:
                nc.vector.reciprocal_approx_fast(out=dinv[:], in_=dsb[:])
            else:
                nc.vector.reciprocal(dinv[:], dsb[:])

            # normalize tail as a deferred closure: emitted a few chunk
            # iterations into the NEXT sweep, so the Pool-queue broadcasts
            # never sit at the queue head waiting on the reciprocal chain
            # (head-of-line blocking the next sweep's affine_selects).
            # partition_broadcast does not honor a non-zero output base
            # partition on HW (head1 goes via a base-0 temp + copy); DVE
            # TensorTensor needs equal input base partitions in SBUF.
            def norm_tail(j=j, p=p, jsl=jsl, dinv=dinv):
                rbf = stage.tile([P, TT], F32, tag="rbf", name=f"rbf_{j}_{p}")
                nc.gpsimd.partition_broadcast(rbf[0:DH, :], dinv[0:1, 0, :])
                rbt = stage.tile([DH, TT], F32, tag="rbt", name=f"rbt_{j}_{p}")
                nc.gpsimd.partition_broadcast(rbt[:], dinv[0:1, 1, :])
                nc.vector.tensor_copy(rbf[DH:P, :], rbt[:])
                nc.vector.tensor_mul(attnT[:, p, jsl], attnT[:, p, jsl],
                                     rbf[:])
            work.appendleft(norm_tail)

        # tile 0: pair-0 q/k projections immediately (all attn(0) pair 0
        # needs); pair-1 + V closures drain at the first chunk so scoring
        # starts as soon as possible. xt1 is staged up front too (its DMA
        # shares the startup window), proj(1) drains during attn(0).
        vwork = {}
        qk0, vwork[0] = make_proj(0, xt0)
        for f in qk0:
            f()
        xt1 = xpool.tile([P, KD, TT], BF16, tag="xt", name="xt_1")
        for c in range(KD):
            nc.gpsimd.dma_start(xt1[:, c, :], xT[c, 1])
        qk1, vwork[1] = make_proj(1, xt1)
        work.extend(qk1)

        for j in range(NT):
            ncb = 4 * (j + 1)
            # stage + enqueue tile j+1 at phase start (xt1 staged upfront);
            # the ration spreads its closures over BOTH pair sweeps, with a
            # few skipped iterations up front so drained matmuls never wait
            # on the just-issued xt DMA in the in-order PE queue.
            if 1 <= j and j + 1 < NT:
                xt = xpool.tile([P, KD, TT], BF16, tag="xt", name=f"xt_{j+1}")
                for c in range(KD):
                    nc.gpsimd.dma_start(xt[:, c, :], xT[c, j + 1])
                nxtqk, vwork[j + 1] = make_proj(j + 1, xt)
                # q/k closures gate attn(j+1)'s start — drain them FIRST
                # (ahead of leftover W_O work); V drains inside pair 0
                work.extendleft(reversed(nxtqk))

            # attn(0) pair 0 runs while xt1/wv still stream: only the V
            # closures (explicit drain) go in; everything else would block
            # the in-order PE queue behind unarrived DMAs. Later phases
            # spread the drains over both sweeps, skipping a few iterations
            # so drained matmuls never wait on the just-issued xt DMA.
            set_ration(2 * ncb if j > 0 else 10**9, skip=4 if j > 0 else 0)
            pair_sweep(j, 0)
            if j == 0:
                set_ration(ncb)
            pair_sweep(j, 1)
            drain(len(work))   # proj(j+1) fully emitted before attn(j+1)
            work.extend(make_wo(j))
        drain(len(work))


_NC_CACHE = None


def _get_nc():
    global _NC_CACHE
    if _NC_CACHE is None:
        nc = bacc.Bacc("TRN2", target_bir_lowering=False, debug=False,
                       num_devices=N_CORES)
        with tile.TileContext(nc) as tc:
            _body(tc)
        nc.compile()
        _NC_CACHE = nc
    return _NC_CACHE


def _in_maps(x, W_Q, W_K, W_V, W_O):
    import ml_dtypes
    bf = ml_dtypes.bfloat16
    x32 = np.asarray(x, dtype=np.float32)
    W_Q = np.asarray(W_Q, dtype=np.float32)
    W_K = np.asarray(W_K, dtype=np.float32)
    W_V = np.asarray(W_V, dtype=np.float32)
    W_O = np.asarray(W_O, dtype=np.float32)
    # chunk-major layouts so every device DMA is a contiguous block
    xTb = [np.ascontiguousarray(
        x32[b].T.reshape(KD, P, NT, TT).transpose(0, 2, 1, 3)).astype(bf)
        for b in range(B)]
    maps = []
    for core in range(N_CORES):
        hq, b = core // 2, core % 2
        sl = slice(HD * hq, HD * hq + HD)
        maps.append({
            "xT": xTb[b],
            "wq": np.ascontiguousarray(W_Q[:, sl].reshape(KD, P, HD)).astype(bf),
            "wk": np.ascontiguousarray(W_K[:, sl].reshape(KD, P, HD)).astype(bf),
            "wv": np.ascontiguousarray(W_V[:, sl].reshape(KD, P, HD)).astype(bf),
            "wo": np.ascontiguousarray(
                W_O[sl, :].reshape(NP, P, D).transpose(1, 0, 2)).astype(bf),
        })
    return maps


def _gather(results):
    acc = np.zeros([B, D, S], np.float32)
    for core, r in enumerate(results):
        hq, b = core // 2, core % 2
        acc[b] += r["outT"].astype(np.float32).transpose(
            0, 2, 1, 3).reshape(D, S)
    return np.ascontiguousarray(acc.transpose(0, 2, 1))


def kernel(x, W_Q, W_K, W_V, W_O):
    nc = _get_nc()
    res = run_bass_kernel_spmd(nc, _in_maps(x, W_Q, W_K, W_V, W_O),
                               core_ids=list(range(N_CORES)))
    return _gather(res.results)


def kernel_profiled(x, W_Q, W_K, W_V, W_O):
    """Like kernel() but with NTFF tracing; returns (output, exec_time_ns)."""
    nc = _get_nc()
    res = run_bass_kernel_spmd(nc, _in_maps(x, W_Q, W_K, W_V, W_O),
                               core_ids=list(range(N_CORES)), trace=True)
    return _gather(res.results), res.exec_time_ns


# revision 6
# speedup vs baseline: 1.0623x; 1.0041x over previous
"""Causal multi-head attention on 8 Trainium2 NeuronCores.

Problem: x[2,2048,1024] @ W_Q/K/V[1024,1024] -> 16-head causal attention
(d_head=64) -> @ W_O[1024,1024].

Sharding: core c = (hq, b) with hq = c//2, b = c%2. Core owns heads
4hq..4hq+3 (columns [256hq:256hq+256) of W_Q/K/V, rows of W_O) and batch b.
Each core computes a partial output [1024, 2048] (transposed, bf16); the
host sums the 4 head-quad partials per batch and un-transposes.

Design (per core, bf16 operands, fp32 PSUM):
  - Fused pipeline over 4 token tiles of 512: project tile t, causal
    attention for q-tile j=t (its K/V chunks are all ready), W_O for tile
    j - with projection/W_O matmuls of neighboring tiles rationed into
    the in-order PE queue between attention chunks so the PE stays busy
    (HAM stays warm) while the scalar engine (exp) paces the steady state.
  - Q/K projected transposed ([dims, tokens]); V projected directly in
    natural [token, dim] layout (lhsT = x chunk) - no PE transposes.
  - Heads packed in pairs on partitions 0-63 / 64-127; the two score
    matmuls of a pair run on disjoint PE row groups (concurrent on HW).
  - One exp per (pair, chunk) over [128, 2, 512] straddling two PSUM
    banks; causal band masking via gpsimd affine_select on the probs.
  - PV accumulates [96, 512] per head (64 dims + ones column + pad; the
    ones column yields the softmax denominator for free). Reciprocal via
    the fast custom DVE op on base-partition-0 staging tiles; the
    normalize broadcasts+multiply are deferred into the next sweep so
    they never head-of-line-block the Pool queue.
  - All DRAM layouts chunk-major so every DMA moves contiguous blocks.
PSUM: scores 2x[128,2,512] (4 banks) + PV [96,2,512] (2) + proj/WO ring
2x[128,512] (2) = 8 banks.

Measured: 377us (fp32r phase-serial baseline) -> ~181us, rel err 3.9e-3.
"""

from collections import deque

import numpy as np

import concourse.bass as bass
import concourse.tile as tile
from concourse import bacc, mybir
from concourse.bass_utils import run_bass_kernel_spmd

F32 = mybir.dt.float32
BF16 = mybir.dt.bfloat16

N_CORES = 8
P = 128

# HW-bisection flags (CoreSim passes all combinations; HW may not)
USE_STRIDED_EXP = True    # one exp over [128, 2, 512] vs per-head 2D exps
USE_AFFSEL = True         # causal mask via gpsimd.affine_select on probs
USE_FAST_RECIP = True     # reciprocal_approx_fast direct from PSUM
XPOOL_BUFS = 4            # x-tile ring depth; 2 races on HW (DMA vs PE reads)
D = 1024          # d_model
B = 2             # batch
S = 2048          # seq len per core (one batch)
TT = 512          # token tile (free dim of matmuls)
NT = S // TT      # 4 token tiles
KD = D // P       # 8 contraction chunks for projections
NCH = S // P      # 16 k-chunks
HD = 256          # head dims per core (4 heads x 64)
DH = 64           # head dim
NP = 2            # head pairs per core
VW = 96           # V block width: 64 dims + ones col + pad to a 32-multiple


def _body(tc):
    nc = tc.nc
    # all DRAM layouts are chunk-major so every DMA moves a contiguous
    # [128, *] block (strided descriptors halve DMA throughput)
    xT = nc.dram_tensor("xT", [KD, NT, P, TT], BF16, kind="ExternalInput").ap()
    wq = nc.dram_tensor("wq", [KD, P, HD], BF16, kind="ExternalInput").ap()
    wk = nc.dram_tensor("wk", [KD, P, HD], BF16, kind="ExternalInput").ap()
    wv = nc.dram_tensor("wv", [KD, P, HD], BF16, kind="ExternalInput").ap()
    wo = nc.dram_tensor("wo", [P, NP, D], BF16, kind="ExternalInput").ap()
    outT = nc.dram_tensor("outT", [KD, NT, P, TT], BF16,
                          kind="ExternalOutput").ap()

    import contextlib
    with contextlib.ExitStack() as ctx:
        wpool = ctx.enter_context(tc.tile_pool(name="wpool", bufs=1))
        xpool = ctx.enter_context(tc.tile_pool(name="xpool", bufs=XPOOL_BUFS))
        persist = ctx.enter_context(tc.tile_pool(name="persist", bufs=1))
        prp = ctx.enter_context(tc.tile_pool(name="probs", bufs=8))
        stage = ctx.enter_context(tc.tile_pool(name="stage", bufs=3))
        obp = ctx.enter_context(tc.tile_pool(name="obp", bufs=3))
        ps_sc = ctx.enter_context(tc.tile_pool(name="ps_sc", bufs=2, space="PSUM"))
        ps_pv = ctx.enter_context(tc.tile_pool(name="ps_pv", bufs=1, space="PSUM"))
        ps_pw = ctx.enter_context(tc.tile_pool(name="ps_pw", bufs=2, space="PSUM"))

        # mask_band[k, q] = 1.0 if q >= k else 0.0 (only for USE_AFFSEL=False)
        mask_band = None
        if not USE_AFFSEL:
            mask_band = wpool.tile([P, P], F32)
            nc.any.memset(mask_band[:], 1.0)
            nc.gpsimd.affine_select(
                out=mask_band[:], in_=mask_band[:],
                compare_op=mybir.AluOpType.is_ge,
                fill=0.0, base=0, pattern=[[1, P]],
                channel_multiplier=-1)

        # --- PE warmup: dummy matmuls keep the HAM clock-gate warm while
        # the initial DMAs stream in (no data deps, results discarded)
        dum = wpool.tile([P, TT], BF16)
        nc.vector.memset(dum[:], 0.0)
        dps = ps_pw.tile([P, TT], F32, tag="pw", name="warmup")
        for _ in range(14):
            nc.tensor.matmul(dps[:], dum[:, 0:P], dum[:],
                             start=True, stop=True)

        # --- weights (per-chunk DMAs so the first matmuls start early) ----
        xt0 = xpool.tile([P, KD, TT], BF16, tag="xt", name="xt_0")
        wq_sb = wpool.tile([P, KD, HD], BF16)
        wk_sb = wpool.tile([P, KD, HD], BF16)
        wv_sb = wpool.tile([P, KD, HD], BF16)
        for c in range(KD):
            nc.gpsimd.dma_start(xt0[:, c, :], xT[c, 0])
            nc.sync.dma_start(wq_sb[:, c, :], wq[c])
            nc.sync.dma_start(wk_sb[:, c, :], wk[c])
        for c in range(KD):   # wv only feeds the (deferred) V projection
            nc.sync.dma_start(wv_sb[:, c, :], wv[c])
        wo_sb = wpool.tile([P, NP, D], BF16)
        nc.sync.dma_start(wo_sb[:], wo)

        # --- persistent activations ---------------------------------------
        qT = persist.tile([P, NP, S], BF16)    # pair p: head 2p on part 0-63
        kT = persist.tile([P, NP, S], BF16)
        vn = persist.tile([P, NCH, 4, VW], BF16)  # [tok, chunk, head, d|1|pad]
        attnT = persist.tile([P, NP, S], BF16)
        # ones columns via memset (NOT activation(in*0+1): uninitialized SBUF
        # may hold NaN and NaN*0 = NaN on hardware); zero the pad columns
        for l in range(4):
            nc.vector.memset(vn[:, :, l, DH], 1.0)
            nc.vector.memset(vn[:, :, l, DH + 1:VW], 0.0)

        # deferred work: closures emitting ~1-2us of PE work each, drained
        # between attention chunk iterations to keep the PE queue dense.
        # Rationed so the queue lasts the whole attention phase (the PE
        # would otherwise run dry near pair transitions and HAM-rethrottle).
        work = deque()
        ration = [0.0, 0.0]   # step, accumulator

        def set_ration(iters, skip=0):
            ration[0] = len(work) / max(iters - skip, 1)
            ration[1] = -skip * ration[0]

        def drain_step():
            ration[1] += ration[0]
            while ration[1] >= 1.0 and work:
                work.popleft()()
                ration[1] -= 1.0

        def drain(n):
            for _ in range(min(n, len(work))):
                work.popleft()()

        def make_proj(t, xt):
            """Projection closures for token tile t (uses staged xt)."""
            ops = []
            tsl = bass.ts(t, TT)

            def qk_proj(wsb, dstT, p, t=t, xt=xt, tsl=tsl):
                def f():
                    ps = ps_pw.tile([P, TT], F32, tag="pw",
                                    name=f"pj_{wsb is wk_sb}_{t}_{p}")
                    for c in range(KD):
                        nc.tensor.matmul(ps[:], wsb[:, c, bass.ts(p, P)],
                                         xt[:, c, :],
                                         start=(c == 0), stop=(c == KD - 1))
                    nc.vector.tensor_copy(dstT[:, p, tsl], ps[:])
                return f

            def v_proj(tc0, t=t, xt=xt):
                def f():
                    ps = ps_pw.tile([P, 2, HD], F32, tag="pw",
                                    name=f"pv_{t}_{tc0}")
                    for i in range(2):
                        tci = tc0 + i
                        for c in range(KD):
                            nc.tensor.matmul(
                                ps[:, i, :],
                                xt[:, c, bass.ts(tci, P)],
                                wv_sb[:, c, :],
                                start=(c == 0), stop=(c == KD - 1))
                    for i in range(2):
                        ch = t * 4 + tc0 + i
                        nc.vector.tensor_copy(
                            vn[:, ch, :, 0:DH],
                            ps[:, i, :].rearrange("p (h d) -> p h d", h=4))
                return f

            for p in range(NP):
                ops.append(qk_proj(wq_sb, qT, p))
                ops.append(qk_proj(wk_sb, kT, p))
            return ops, [v_proj(0), v_proj(2)]

        def make_wo(j):
            """Output projection closures for q-tile j (reads attnT)."""
            ops = []
            jsl = bass.ts(j, TT)
            # the last tile's W_O runs after attention ends: borrow the
            # score-PSUM ring (free by then) for 2x the groups in flight
            wpool_ps = ps_sc if j == NT - 1 else ps_pw
            wtag = "sc" if j == NT - 1 else "pw"
            for f_ in range(KD):
                def g(f_=f_, jsl=jsl, j=j):
                    ps = wpool_ps.tile([P, TT], F32, tag=wtag,
                                       name=f"wo_{j}_{f_}")
                    for a in range(NP):
                        nc.tensor.matmul(ps[:], wo_sb[:, a, bass.ts(f_, P)],
                                         attnT[:, a, jsl],
                                         start=(a == 0), stop=(a == NP - 1))
                    ob = obp.tile([P, TT], BF16, tag="ob", name=f"ob_{j}_{f_}")
                    if j == NT - 1 and f_ % 2:
                        # tail: ACT is idle after the last exp — share the
                        # PSUM drain copies between the two engines
                        nc.scalar.activation(
                            ob[:], ps[:], mybir.ActivationFunctionType.Copy)
                    else:
                        nc.vector.tensor_copy(ob[:], ps[:])
                    nc.sync.dma_start(outT[f_, j], ob[:])
                ops.append(g)
            return ops

        def pair_sweep(j, p, prelude=None):
            """Scores+exp+mask+PV chunk loop for (q-tile j, head pair p).
            Returns a `post` closure (final PV, PSUM drain copies,
            reciprocal, deferred normalize) which the caller may pass as
            the next sweep's `prelude`: it is then emitted after the next
            sweep's first two chunks, so the exp stream never waits for a
            pair tail. The PV PSUM tile is allocated lazily (at the first
            PV matmul, after the prelude ran) to keep the 1-deep ring's
            release emission ahead of the next allocation."""
            jsl = bass.ts(j, TT)
            ncb = 4 * (j + 1)
            dinv = stage.tile([1, NP, TT], F32, tag=f"dinv{p}",
                              name=f"dinv_{j}_{p}")
            box = {}

            def pvh(h):
                if "pv" not in box:
                    box["pv"] = ps_pv.tile([VW, NP, TT], F32, tag="pv",
                                           name=f"pv_{j}_{p}")
                return box["pv"][:, h, :]

            pending = None
            for cb in range(ncb):
                r = cb - 4 * j
                lo = P * r if r > 0 else 0
                csl = bass.ts(cb, P)
                sc = ps_sc.tile([P, NP, TT], F32, tag="sc",
                                name=f"sc_{j}_{p}_{cb}")
                for h in range(NP):
                    hp = slice(DH * h, DH * h + DH)
                    nc.tensor.matmul(sc[:, h, lo:], kT[hp, p, csl],
                                     qT[hp, p, jsl][:, lo:],
                                     start=True, stop=True)
                pr = prp.tile([P, NP, TT], BF16, tag="pr",
                              name=f"pr_{j}_{p}_{cb}")
                if USE_STRIDED_EXP:
                    nc.scalar.activation(pr[:, :, lo:], sc[:, :, lo:],
                                         mybir.ActivationFunctionType.Exp,
                                         scale=0.125)
                else:
                    for h in range(NP):
                        nc.scalar.activation(
                            pr[:, h, lo:], sc[:, h, lo:],
                            mybir.ActivationFunctionType.Exp, scale=0.125)
                if r >= 0:
                    rsl = bass.ts(r, P)
                    for h in range(NP):
                        if USE_AFFSEL:
                            nc.gpsimd.affine_select(
                                out=pr[:, h, rsl], in_=pr[:, h, rsl],
                                compare_op=mybir.AluOpType.is_ge,
                                fill=0.0, base=0, pattern=[[1, P]],
                                channel_multiplier=-1)
                        else:
                            nc.vector.tensor_mul(pr[:, h, rsl],
                                                 pr[:, h, rsl],
                                                 mask_band[:])
                if cb == 1 and prelude is not None:
                    prelude()
                    prelude = None
                if pending is not None:
                    pcb, ppr, plo = pending
                    for h in range(NP):
                        nc.tensor.matmul(
                            pvh(h)[:, plo:],
                            vn[:, pcb, 2 * p + h, :], ppr[:, h, plo:],
                            start=(pcb == 0), stop=False)
                pending = (cb, pr, lo)
                if p == 0 and vwork.get(j):
                    if cb in ((0, 1) if j == 0 else (2, 3)):
                        vwork[j].pop(0)()
                drain_step()

            def post(pending=pending):
                pcb, ppr, plo = pending
                for h in range(NP):
                    nc.tensor.matmul(pvh(h)[:, plo:],
                                     vn[:, pcb, 2 * p + h, :],
                                     ppr[:, h, plo:],
                                     start=(pcb == 0), stop=True)
                # free the PV banks: unnormalized copy + denominator recip.
                # For the very last pair, ACT is idle (no exps left).
                tail = (j == NT - 1 and p == NP - 1)
                dsb = stage.tile([1, NP, TT], F32, tag=f"dsb{p}",
                                 name=f"dsb_{j}_{p}")
                for h in range(NP):
                    if tail:
                        nc.scalar.activation(
                            attnT[bass.ts(h, DH), p, jsl], pvh(h)[0:DH, :],
                            mybir.ActivationFunctionType.Copy)
                    else:
                        nc.vector.tensor_copy(attnT[bass.ts(h, DH), p, jsl],
                                              pvh(h)[0:DH, :])
                    nc.vector.tensor_copy(dsb[0:1, h, :],
                                          pvh(h)[DH:DH + 1, :])
                if USE_FAST_RECIP:
                    nc.vector.reciprocal_approx_fast(out=dinv[:], in_=dsb[:])
                else:
                    nc.vector.reciprocal(dinv[:], dsb[:])

                def norm_tail():
                    rbf = stage.tile([P, TT], F32, tag="rbf",
                                     name=f"rbf_{j}_{p}")
                    nc.gpsimd.partition_broadcast(rbf[0:DH, :],
                                                  dinv[0:1, 0, :])
                    rbt = stage.tile([DH, TT], F32, tag="rbt",
                                     name=f"rbt_{j}_{p}")
                    nc.gpsimd.partition_broadcast(rbt[:], dinv[0:1, 1, :])
                    nc.vector.tensor_copy(rbf[DH:P, :], rbt[:])
                    nc.vector.tensor_mul(attnT[:, p, jsl],
                                         attnT[:, p, jsl], rbf[:])
                work.appendleft(norm_tail)
            return post

        # tile 0: pair-0 q/k projections immediately (all attn(0) pair 0
        # needs); pair-1 + V closures drain at the first chunk so scoring
        # starts as soon as possible. xt1 is staged up front too (its DMA
        # shares the startup window), proj(1) drains during attn(0).
        vwork = {}
        qk0, vwork[0] = make_proj(0, xt0)
        for f in qk0:
            f()
        xt1 = xpool.tile([P, KD, TT], BF16, tag="xt", name="xt_1")
        for c in range(KD):
            nc.gpsimd.dma_start(xt1[:, c, :], xT[c, 1])
        qk1, vwork[1] = make_proj(1, xt1)
        work.extend(qk1)

        for j in range(NT):
            ncb = 4 * (j + 1)
            # stage + enqueue tile j+1 at phase start (xt1 staged upfront);
            # the ration spreads its closures over BOTH pair sweeps, with a
            # few skipped iterations up front so drained matmuls never wait
            # on the just-issued xt DMA in the in-order PE queue.
            if 1 <= j and j + 1 < NT:
                xt = xpool.tile([P, KD, TT], BF16, tag="xt", name=f"xt_{j+1}")
                for c in range(KD):
                    nc.gpsimd.dma_start(xt[:, c, :], xT[c, j + 1])
                nxtqk, vwork[j + 1] = make_proj(j + 1, xt)
                # q/k closures gate attn(j+1)'s start — drain them FIRST
                # (ahead of leftover W_O work); V drains inside pair 0
                work.extendleft(reversed(nxtqk))

            # attn(0) pair 0 runs while xt1/wv still stream: only the V
            # closures (explicit drain) go in; everything else would block
            # the in-order PE queue behind unarrived DMAs. Later phases
            # spread the drains over both sweeps, skipping a few iterations
            # so drained matmuls never wait on the just-issued xt DMA.
            set_ration(2 * ncb if j > 0 else 10**9, skip=4 if j > 0 else 0)
            post0 = pair_sweep(j, 0)
            if j == 0:
                set_ration(ncb)
            pair_sweep(j, 1, prelude=post0)()
            drain(len(work))   # proj(j+1) fully emitted before attn(j+1)
            work.extend(make_wo(j))
        drain(len(work))


_NC_CACHE = None


def _get_nc():
    global _NC_CACHE
    if _NC_CACHE is None:
        nc = bacc.Bacc("TRN2", target_bir_lowering=False, debug=False,
                       num_devices=N_CORES)
        with tile.TileContext(nc) as tc:
            _body(tc)
        nc.compile()
        _NC_CACHE = nc
    return _NC_CACHE


def _in_maps(x, W_Q, W_K, W_V, W_O):
    import ml_dtypes
    bf = ml_dtypes.bfloat16
    x32 = np.asarray(x, dtype=np.float32)
    W_Q = np.asarray(W_Q, dtype=np.float32)
    W_K = np.asarray(W_K, dtype=np.float32)
    W_V = np.asarray(W_V, dtype=np.float32)
    W_O = np.asarray(W_O, dtype=np.float32)
    # chunk-major layouts so every device DMA is a contiguous block
    xTb = [np.ascontiguousarray(
        x32[b].T.reshape(KD, P, NT, TT).transpose(0, 2, 1, 3)).astype(bf)
        for b in range(B)]
    maps = []
    for core in range(N_CORES):
        hq, b = core // 2, core % 2
        sl = slice(HD * hq, HD * hq + HD)
        maps.append({
            "xT": xTb[b],
            "wq": np.ascontiguousarray(W_Q[:, sl].reshape(KD, P, HD)).astype(bf),
            "wk": np.ascontiguousarray(W_K[:, sl].reshape(KD, P, HD)).astype(bf),
            "wv": np.ascontiguousarray(W_V[:, sl].reshape(KD, P, HD)).astype(bf),
            "wo": np.ascontiguousarray(
                W_O[sl, :].reshape(NP, P, D).transpose(1, 0, 2)).astype(bf),
        })
    return maps


def _gather(results):
    acc = np.zeros([B, D, S], np.float32)
    for core, r in enumerate(results):
        hq, b = core // 2, core % 2
        acc[b] += r["outT"].astype(np.float32).transpose(
            0, 2, 1, 3).reshape(D, S)
    return np.ascontiguousarray(acc.transpose(0, 2, 1))


def kernel(x, W_Q, W_K, W_V, W_O):
    nc = _get_nc()
    res = run_bass_kernel_spmd(nc, _in_maps(x, W_Q, W_K, W_V, W_O),
                               core_ids=list(range(N_CORES)))
    return _gather(res.results)


def kernel_profiled(x, W_Q, W_K, W_V, W_O):
    """Like kernel() but with NTFF tracing; returns (output, exec_time_ns)."""
    nc = _get_nc()
    res = run_bass_kernel_spmd(nc, _in_maps(x, W_Q, W_K, W_V, W_O),
                               core_ids=list(range(N_CORES)), trace=True)
    return _gather(res.results), res.exec_time_ns
